# revision 31
# baseline (speedup 1.0000x reference)
"""Trainium2 Bass kernel for nn_EquivariantPerturbationTransform.

Reference (N=6000 genes, D=256, H=8 heads, P=128 perturbations, B=16):
  q = H @ Wq.T ; k,v from gathered perturbation rows
  scores[h,n,p] shared across batches; per-batch mask over p (ragged)
  attn_out[b] = softmax-masked attention -> out proj (zeroed for empty b)
  x = LN1(H + attn_out); out = LN2(x + gelu(x@W1.T)@W2.T)

v3 strategy (sequence-parallel over 8 cores, 768 query rows/core):
  - Wo folded into values on the host (vo), so the attention context IS the
    projected attn_out (as v2).
  - Normalized attention weights Ehatall[(h,p16), n] = E/den computed ONCE
    for all batches (den from Et directly via per-head mask stationaries;
    replication matmul rep_all broadcasts each row's own batch denominator).
    Per (batch, group) the stationary is a row-masked copy (one DVE
    tensor_scalar with a 0/1 mask column); pure single-batch groups use
    Ehatall directly.
  - E head-layout -> block-layout regroup via a DRAM round-trip (8 writes +
    1 readback) instead of 64 SBUF-SBUF DMA triggers.
  - H residual and FFN residual moved off the PE onto Pool (tensor_tensor
    adds from PSUM); LN stats on DVE from bf16 SBUF tensors.
  - FFN2 computed in ROW layout (stationary = gelu-output slices, moving =
    W2 row-major k-tiles) so its output lands where LN2 needs it: no yT
    evac and no output transposes.
  - PE p-state: the tensor engine doubles to ~2.4GHz after ~3us of gap-free
    execution. Emission orders the PE queue [xpose | FFN1(m0-3) | ctx |
    FFN1(m4-7) | FFN2] per step with all dependencies scheduled to land
    before the PE reaches them, and PSUM pools sized so no matmul ever
    waits on an evacuation.
  - rsqrt for the LNs via quadratic seed + Newton step on DVE/Pool so the
    ACT table never leaves gelu after the phase-A Exp.
"""

import os
import sys

sys.path.insert(0, "/opt/trn_rl_repo")

import numpy as np

import concourse.bass as bass
from concourse import mybir
from concourse.tile import TileContext

F32 = mybir.dt.float32
F32R = mybir.dt.float32r
BF16 = mybir.dt.bfloat16
AF = mybir.ActivationFunctionType
ALU = mybir.AluOpType

N, D, H, P, B = 6000, 256, 8, 128, 16
DH = D // H  # 32
NCORES = 8
NPAD = 6144
NG = NPAD // NCORES  # 768 rows per core
NT = NG // 128       # 6 row tiles
EPS = 1e-5
GW = 16              # perturbation block width
NGRP = P // GW       # 8 blocks
F1 = 4 * D           # 1024


def _split_waits(nc, max_waits=1):
    """The neuronxcc/walrus build here rejects >1 sync-wait per instruction;
    hoist excess waits onto same-engine NoOps (semantically identical)."""
    n_split = 0
    for f in nc.m.functions:
        for bb in f.blocks:
            new_list = []
            for ins in bb.instructions:
                si = getattr(ins, "sync_info", None)
                if si is not None and si.on_wait and len(si.on_wait) > max_waits:
                    waits = list(si.on_wait)
                    excess, keep = waits[:-max_waits], waits[-max_waits:]
                    for i in range(0, len(excess), max_waits):
                        chunk = excess[i : i + max_waits]
                        nop = mybir.InstNoOp(name=f"{ins.name}-ws{i}", ins=[], outs=[])
                        nop.engine = ins.engine
                        nop.sync_info = mybir.SyncInfo(on_wait=chunk, on_update=[])
                        new_list.append(nop)
                        n_split += 1
                    si.on_wait = keep
                new_list.append(ins)
            bb.instructions = new_list
    return n_split


def _build_program(counts, blocks, npair, n_early, flags):
    """blocks[b] = list of (g, pair_idx) block descriptors; groups < n_early
    are regrouped with direct per-(g,h) DMAs, the rest via DRAM staging."""
    (use_bq, use_b1, use_b2, use_g1, use_b1ln, use_g2, use_b2ln) = flags
    nc = bass.Bass()

    # ---- DRAM parameters (already in on-chip [128, ...] layouts) ---------
    hgb = nc.declare_dram_parameter("hgb", [128, NT, D], F32, isOutput=False)
    hgt = nc.declare_dram_parameter("hgt", [128, 2, NG], BF16, isOutput=False)
    kt = nc.declare_dram_parameter("kt", [128, 2, P], F32R, isOutput=False)
    wqt = nc.declare_dram_parameter("wqt", [128, 2, D], BF16, isOutput=False)
    bq_col = nc.declare_dram_parameter("bq_col", [128, 2, 1], F32, isOutput=False)
    vo = nc.declare_dram_parameter("vo", [128, NGRP, D], BF16, isOutput=False)
    m01h = nc.declare_dram_parameter("m01h", [128, H, 128], BF16, isOutput=False)
    repm = nc.declare_dram_parameter("repm", [128, max(npair, 1), 128], BF16, isOutput=False)
    identb = nc.declare_dram_parameter("identb", [128, 128], BF16, isOutput=False)
    emptyp = nc.declare_dram_parameter("emptyp", [128, 1], F32, isOutput=False)
    w1t = nc.declare_dram_parameter("w1t", [128, 2, F1], BF16, isOutput=False)
    w2r = nc.declare_dram_parameter("w2r", [128, 8, D], BF16, isOutput=False)
    b1_col = nc.declare_dram_parameter("b1_col", [128, 8, 1], F32, isOutput=False)
    ln1c = nc.declare_dram_parameter("ln1c", [128, 2, 2], F32, isOutput=False)
    gb_row = nc.declare_dram_parameter("gb_row", [6, D], F32, isOutput=False)
    out = nc.declare_dram_parameter("out", [B, NG, D], F32, isOutput=True)

    s_attn = 1.0 / float(np.sqrt(DH))
    any_empty = any(int(c) == 0 for c in counts)
    jobs = [b for b in range(B) if int(counts[b]) > 0]

    with TileContext(nc) as tc, nc.allow_low_precision(
            reason="bf16 matmul inputs; tolerance budget is 2e-2 of max"):
        import contextlib
        import itertools

        _dma_engines = itertools.cycle([nc.sync, nc.gpsimd])

        def dma(out_ap, in_ap):
            next(_dma_engines).dma_start(out=out_ap, in_=in_ap)

        cstack = contextlib.ExitStack()
        consts = cstack.enter_context(tc.tile_pool(name="consts", bufs=1))
        dramp = cstack.enter_context(tc.tile_pool(name="dramp", bufs=1, space="DRAM"))

        # ---- persistent constants (few, bundled DMAs; critical first) ----
        identb_sb = consts.tile([128, 128], BF16, tag="identb", name="identb_sb")
        dma(identb_sb[:], identb[:, :])
        hgt_sb = consts.tile([128, 2, NG], BF16, tag="hgt", name="hgt_sb")
        dma(hgt_sb[:], hgt[:, :, :])
        wq_sb = consts.tile([128, 2, D], BF16, tag="wq", name="wq_sb")
        dma(wq_sb[:], wqt[:, :, :])
        kt_sb = consts.tile([128, 2, P], F32R, tag="kt", name="kt_sb")
        dma(kt_sb[:], kt[:, :, :])
        hgb_sb = consts.tile([128, NT, D], F32, tag="hgb", name="hgb_sb")
        dma(hgb_sb[:], hgb[:, :, :])
        w1_sb = consts.tile([128, 2, F1], BF16, tag="w1", name="w1_sb")
        dma(w1_sb[:], w1t[:, :, :])
        w2_sb = consts.tile([128, 8, D], BF16, tag="w2", name="w2_sb")
        dma(w2_sb[:], w2r[:, :, :])
        vo_sb = consts.tile([128, NGRP, D], BF16, tag="vo", name="vo_sb")
        dma(vo_sb[:], vo[:, :, :])
        m01_sb = consts.tile([128, H, 128], BF16, tag="m01", name="m01_sb")
        dma(m01_sb[:], m01h[:, :, :])
        rep_sb = consts.tile([128, max(npair, 1), 128], BF16, tag="rep", name="rep_sb")
        dma(rep_sb[:], repm[:, :, :])
        empty_sb = consts.tile([128, 1], F32, tag="empty", name="empty_sb")
        dma(empty_sb[:], emptyp[:, :])
        b1_sb = consts.tile([128, 8, 1], F32, tag="b1c", name="b1_sb")
        dma(b1_sb[:], b1_col[:, :, :])
        bq_sb = None
        if use_bq:
            bq_sb = consts.tile([128, 2, 1], F32, tag="bqc", name="bq_sb")
            dma(bq_sb[:], bq_col[:, :, :])
        ln1_sb = None
        if use_g1 or use_b1ln:
            ln1_sb = consts.tile([128, 2, 2], F32, tag="ln1c", name="ln1_sb")
            dma(ln1_sb[:], ln1c[:, :, :])
        gbr_sb = None
        if use_g2 or use_b2ln or use_b2 or use_g1 or use_b1ln:
            gbr_sb = consts.tile([128, 6, D], F32, tag="gbr", name="gbr_sb")
            nc.gpsimd.dma_start(out=gbr_sb[:], in_=gb_row[:, :].to_broadcast((128, 6, D)))

        # persistent activations
        qT_sb = consts.tile([128, 2, NG], F32R, tag="qT", name="qT_sb")
        Et = consts.tile([128, H, NG], BF16, tag="Et", name="Et")
        Eg = consts.tile([128, NGRP, NG], BF16, tag="Eg", name="Eg")
        den_sb = consts.tile([128, NG], BF16, tag="den", name="den_sb")
        denf_sb = consts.tile([128, NG], F32, tag="denf", name="denf_sb")
        denr_sb = consts.tile([128, NG], BF16, tag="denr", name="denr_sb")
        n_stage = NGRP - n_early
        stg = dramp.tile([max(n_stage, 1), H, GW, NG], BF16, tag="stg", name="stg")

        NCH = 2
        CH = NG // NCH  # 384

        # ================= Phase A: shared projections ==================
        with tc.tile_pool(name="psA", bufs=1, space="PSUM") as psA:
            # PE warmup: ramp the tensor-engine p-state while DMAs land
            wtile = psA.tile([128, NG], BF16, tag="wt", name="wtile")
            for w in range(12):
                nc.tensor.transpose(wtile[:, 0:128], identb_sb[:], identb_sb[:])

            # qT [D, NG] = Wq^T Hg^T  (PSUM writes split at the bank edge)
            for m in range(2):
                ps = psA.tile([128, NG], F32, tag=f"qs{m % 2}", name="ps_q")
                for kk in range(2):
                    for lo, hi in ((0, 512), (512, NG)):
                        nc.tensor.matmul(
                            ps[:, lo:hi], wq_sb[:, kk, m * 128 : (m + 1) * 128],
                            hgt_sb[:, kk, lo:hi], start=(kk == 0), stop=(kk == 1))
                if use_bq:
                    nc.scalar.activation(qT_sb[:, m, :], ps[:], AF.Identity,
                                         bias=bq_sb[:, m, 0:1])
                else:
                    nc.scalar.activation(qT_sb[:, m, :], ps[:], AF.Copy)

            # Et[p, h, n] = exp(s * k_h q_h^T) per head, with the den
            # accumulation matmuls and the early-group regroup DMAs
            # interleaved so the PE never drains during the exp chain.
            psd = [psA.tile([128, CH], F32, tag=f"dn{c}", name=f"psd{c}")
                   for c in range(NCH)]

            def den_mm(h):
                for c in range(NCH):
                    nc.tensor.matmul(
                        psd[c][:], m01_sb[:, h, :],
                        Et[:, h, c * CH : (c + 1) * CH],
                        start=(h == 0), stop=(h == H - 1))

            for h in range(H):
                ps = psA.tile([128, NG], F32, tag=f"qs{h % 2}", name="ps_s")
                for lo, hi in ((0, 512), (512, NG)):
                    nc.tensor.matmul(
                        ps[:, lo:hi],
                        kt_sb[(h % 4) * DH : (h % 4 + 1) * DH, h // 4, :],
                        qT_sb[(h % 4) * DH : (h % 4 + 1) * DH, h // 4, lo:hi],
                        start=True, stop=True,
                        tile_position=((h % 4) * DH, 0))
                nc.scalar.activation(Et[:, h, :], ps[:], AF.Exp, scale=s_attn)
                if h >= 1:
                    den_mm(h - 1)
                for w in range(6):
                    nc.tensor.transpose(wtile[:, 0:128], identb_sb[:],
                                        identb_sb[:])
                for g in range(n_early):
                    dma(Eg[h * GW : (h + 1) * GW, g, :],
                        Et[g * GW : (g + 1) * GW, h, :])
            den_mm(H - 1)

            # den -> reciprocal (fast-approx) -> bf16
            for c in range(NCH):
                nc.vector.tensor_scalar(
                    out=den_sb[:, c * CH : (c + 1) * CH], in0=psd[c][:],
                    scalar1=empty_sb[:, 0:1], scalar2=None, op0=ALU.add)
                nc.vector.reciprocal(
                    out=denr_sb[:, c * CH : (c + 1) * CH],
                    in_=den_sb[:, c * CH : (c + 1) * CH])

            # hold the PE p-state while the reciprocal chain runs
            for w in range(44):
                nc.tensor.transpose(wtile[:, 0:128], identb_sb[:], identb_sb[:])

            # late groups: Et -> DRAM staging -> Eg block layout
            for si in range(n_stage):
                g = n_early + si
                dma(stg[si].rearrange("h i n -> i h n"),
                    Et[g * GW : (g + 1) * GW, :, :])
            if n_stage:
                nc.sync.dma_start(
                    out=Eg[:, n_early:NGRP, :],
                    in_=stg[:].rearrange("g h i n -> (h i) g n"))

        # ================= Phase B: per-batch pipeline ==================
        work = cstack.enter_context(tc.tile_pool(name="work", bufs=2))
        ehpool = cstack.enter_context(tc.tile_pool(name="ehp", bufs=6))
        ps_f1 = cstack.enter_context(tc.tile_pool(name="ps_f1", bufs=2, space="PSUM"))
        ps_m = cstack.enter_context(tc.tile_pool(name="ps_m", bufs=2, space="PSUM"))
        ps_y = cstack.enter_context(tc.tile_pool(name="ps_y", bufs=2, space="PSUM"))

        RC2, RC1, RC0 = 0.29333931447269, -1.1711876763158582, 1.8939170369253155

        def rsqrt_group(vv_ap, mu_ap, sc_ap, ng_ap, sA, sB, eng=None):
            """sc = rsqrt(vv), ng = -mu*sc. Quadratic seed + one Newton step.
            All-DVE by default (cross-engine hops cost semaphore latency);
            with eng=gpsimd the tensor_tensor ops go to Pool (immediate
            tensor_scalar is DVE-only in this toolchain) for chains with
            timing slack."""
            e = eng if eng is not None else nc.vector
            nc.vector.tensor_scalar(out=sA, in0=vv_ap, scalar1=RC2,
                                    scalar2=RC1, op0=ALU.mult, op1=ALU.add)
            e.tensor_tensor(out=sA, in0=sA, in1=vv_ap, op=ALU.mult)
            nc.vector.tensor_scalar(out=sA, in0=sA, scalar1=RC0,
                                    scalar2=None, op0=ALU.add)
            e.tensor_tensor(out=sB, in0=sA, in1=sA, op=ALU.mult)
            e.tensor_tensor(out=sB, in0=sB, in1=vv_ap, op=ALU.mult)
            nc.vector.tensor_scalar(out=sB, in0=sB, scalar1=-0.5,
                                    scalar2=1.5, op0=ALU.mult, op1=ALU.add)
            e.tensor_tensor(out=sc_ap, in0=sA, in1=sB, op=ALU.mult)
            nc.vector.scalar_tensor_tensor(
                out=ng_ap, in0=mu_ap, scalar=-1.0, in1=sc_ap,
                op0=ALU.mult, op1=ALU.mult)

        def new_state(b):
            return {
                "b": b,
                "mvb": work.tile([128, NT, 2], F32, tag="mvb", name="mvb"),
                "sc1": work.tile([128, NT], F32, tag="sc1", name="sc1"),
                "ng1": work.tile([128, NT], F32, tag="ng1", name="ng1"),
                "stats": work.tile([128, 2, 6], F32, tag="stats", name="stats"),
                "xpre": work.tile([128, NT, D], BF16, tag="xpre", name="xpre"),
                "x_row": work.tile([128, NT, D], BF16, tag="x_row", name="x_row",
                                   bufs=3),
                "xT": work.tile([128, 2, NG], BF16, tag="xT", name="xT"),
                "sA": work.tile([128, 6], F32, tag="sA", name="sA"),
                "sB": work.tile([128, 6], F32, tag="sB", name="sB"),
                "eh": [],
            }

        def emit_rep(st):
            """Per-(batch,group) masked replication matmul (PE) + normalize
            multiply (DVE): eh = E_g * masked-replicated reciprocal."""
            b = st["b"]
            if b is None:
                return
            for g, ri in blocks[b]:
                tl = ehpool.tile([128, NG], BF16, tag="eh", name="eh")
                for c in range(NCH):
                    psr = ps_y.tile([128, CH], F32, tag="y", name="psr")
                    nc.tensor.matmul(
                        psr[:], rep_sb[:, ri, :],
                        denr_sb[:, c * CH : (c + 1) * CH],
                        start=True, stop=True)
                    nc.vector.tensor_mul(
                        tl[:, c * CH : (c + 1) * CH],
                        Eg[:, g, c * CH : (c + 1) * CH], psr[:])
                st["eh"].append((g, tl[:]))

        def emit_xp(st):
            """x transposes (PE) + xT evac (split ACT/DVE), per tile."""
            for t in range(NT):
                psx = ps_m.tile([128, D], BF16, tag="m", name="psx")
                for k in range(2):
                    nc.tensor.transpose(
                        psx[:, k * 128 : (k + 1) * 128],
                        st["x_row"][:, t, k * 128 : (k + 1) * 128],
                        identb_sb[:])
                if t < 4:
                    nc.scalar.activation(
                        st["xT"][:, :, t * 128 : (t + 1) * 128],
                        psx[:].rearrange("p (k n) -> p k n", k=2), AF.Copy)
                else:
                    nc.vector.tensor_copy(
                        out=st["xT"][:, :, t * 128 : (t + 1) * 128],
                        in_=psx[:].rearrange("p (k n) -> p k n", k=2))

        def emit_ffn1_m(st, m):
            """One FFN1 m-tile: PE matmuls + ACT gelu."""
            if m == 0:
                st["h1g"] = work.tile([128, 8, NG], BF16, tag="h1g", name="h1g")
            xT = st["xT"]
            ps = ps_f1.tile([128, NG], F32, tag="f1", name="f1")
            for kk in range(2):
                nc.tensor.matmul(ps[:, 0:512], w1_sb[:, kk, m * 128 : (m + 1) * 128],
                                 xT[:, kk, 0:512],
                                 start=(kk == 0), stop=(kk == 1))
                nc.tensor.matmul(ps[:, 512:NG], w1_sb[:, kk, m * 128 : (m + 1) * 128],
                                 xT[:, kk, 512:NG],
                                 start=(kk == 0), stop=(kk == 1))
            if use_b1:
                nc.scalar.activation(st["h1g"][:, m, :], ps[:], AF.Gelu,
                                     bias=b1_sb[:, m, 0:1])
            else:
                nc.scalar.activation(st["h1g"][:, m, :], ps[:], AF.Gelu)

        def emit_front_t(st, t):
            """One front tile: ctx matmuls (PE) + H residual evac (DVE);
            LN1 stats (DVE) in tile pairs."""
            b = st["b"]
            if b is not None:
                psa = ps_m.tile([128, D], F32, tag="m", name="ao")
                nb = len(st["eh"])
                for i, (g, eh_ap) in enumerate(st["eh"]):
                    nc.tensor.matmul(
                        psa[:], eh_ap[:, t * 128 : (t + 1) * 128],
                        vo_sb[:, g, :], start=(i == 0), stop=(i == nb - 1))
                nc.vector.scalar_tensor_tensor(
                    out=st["xpre"][:, t, :], in0=psa[:], scalar=1.0,
                    in1=hgb_sb[:, t, :], op0=ALU.mult, op1=ALU.add)
            src_t = st["xpre"] if b is not None else hgb_sb
            nc.vector.bn_stats(out=st["stats"][:, 0, :], in_=src_t[:, t, :])
            nc.vector.bn_aggr(out=st["mvb"][:, t, :], in_=st["stats"][:, 0, :])

        def emit_front_rsqrt(st, half):
            lo, hi = (0, 2) if half == 0 else (2, 6)
            mvb, sc1, ng1 = st["mvb"], st["sc1"], st["ng1"]
            rsqrt_group(mvb[:, lo:hi, 1], mvb[:, lo:hi, 0],
                        sc1[:, lo:hi], ng1[:, lo:hi],
                        st["sA"][:, lo:hi], st["sB"][:, lo:hi])

        def emit_front_apply(st, half):
            """LN1 applies (Pool tensor_scalar, per-partition AP scalars)."""
            b = st["b"]
            sc1, ng1 = st["sc1"], st["ng1"]
            src = st["xpre"] if b is not None else hgb_sb
            for t in (range(0, 2) if half == 0 else range(2, 6)):
                nc.gpsimd.tensor_scalar(
                    out=st["x_row"][:, t, :], in0=src[:, t, :],
                    scalar1=sc1[:, t : t + 1], scalar2=ng1[:, t : t + 1],
                    op0=ALU.mult, op1=ALU.add)

        def emit_ffn2_ln2(st):
            """FFN2 in row layout (PE, t-outer k-inner) + y residual evac
            (DVE) + LN2 stats (DVE)."""
            st["ypre"] = work.tile([128, NT, D], BF16, tag="ypre", name="ypre")
            st["mv2"] = work.tile([128, NT, 2], F32, tag="mv2", name="mv2")
            st["st2"] = work.tile([128, 2, 6], F32, tag="st2", name="st2")
            h1g = st["h1g"]
            xres = st["x_row"]
            if use_g1 or use_b1ln:
                # residual needs the true x = x_norm*g1 + be1 (g1 folded into
                # W1 elsewhere); rare flag path
                xres = work.tile([128, NT, D], BF16, tag="xres", name="xres")
                for t in range(NT):
                    nc.vector.tensor_mul(xres[:, t, :], st["x_row"][:, t, :],
                                         gbr_sb[:, 4, :])
                    nc.vector.tensor_add(xres[:, t, :], xres[:, t, :],
                                         gbr_sb[:, 5, :])
            for t in range(NT):
                psy = ps_y.tile([128, D], F32, tag="y", name="psy")
                for kk in range(8):
                    nc.tensor.matmul(
                        psy[:], h1g[:, kk, t * 128 : (t + 1) * 128],
                        w2_sb[:, kk, :], start=(kk == 0), stop=(kk == 7))
                if use_b2:
                    nc.vector.tensor_add(psy[:], psy[:], gbr_sb[:, 0, :])
                nc.vector.scalar_tensor_tensor(
                    out=st["ypre"][:, t, :], in0=psy[:], scalar=1.0,
                    in1=xres[:, t, :], op0=ALU.mult, op1=ALU.add)
                nc.vector.bn_stats(out=st["st2"][:, 0, :],
                                   in_=st["ypre"][:, t, :])
                nc.vector.bn_aggr(out=st["mv2"][:, t, :],
                                  in_=st["st2"][:, 0, :])

        def emit_ln2_store(st):
            """LN2 rsqrt (DVE+Pool) + apply (Pool) + store."""
            b = st["b"]
            sc2 = work.tile([128, NT], F32, tag="sc2", name="sc2")
            ng2 = work.tile([128, NT], F32, tag="ng2", name="ng2")
            sA2 = work.tile([128, 6], F32, tag="sA2", name="sA2")
            sB2 = work.tile([128, 6], F32, tag="sB2", name="sB2")
            orow = work.tile([128, NT, D], F32, tag="orow", name="orow")
            ypre, mv2 = st["ypre"], st["mv2"]
            rsqrt_group(mv2[:, :, 1], mv2[:, :, 0], sc2[:, :], ng2[:, :],
                        sA2[:], sB2[:])
            for t in range(NT):
                nc.gpsimd.tensor_scalar(
                    out=orow[:, t, :], in0=ypre[:, t, :],
                    scalar1=sc2[:, t : t + 1], scalar2=ng2[:, t : t + 1],
                    op0=ALU.mult, op1=ALU.add)
                if use_g2:
                    nc.vector.tensor_mul(orow[:, t, :], orow[:, t, :],
                                         gbr_sb[:, 2, :])
                if use_b2ln:
                    nc.vector.tensor_add(orow[:, t, :], orow[:, t, :],
                                         gbr_sb[:, 3, :])
            if b is not None:
                dma(out[b].rearrange("(t p) d -> p t d", p=128), orow[:])
            else:
                for be in range(B):
                    if int(counts[be]) == 0:
                        dma(out[be].rearrange("(t p) d -> p t d", p=128),
                            orow[:])

        # ---- software-pipelined emission --------------------------------
        # PE queue per step: xp(bk) | FFN1-m(bk) interleaved with ctx-t(fr)
        # | FFN2(bk).  ACT queue: gelus(bk) | LN1 applies(fr).  The m/t
        # interleave spaces PSUM-slot reuse past the Pool/DVE drain latency
        # so the PE stream never stalls.
        steps = ([None] if any_empty else []) + jobs
        nsteps = len(steps)
        states = {}
        states[0] = new_state(steps[0])
        emit_rep(states[0])
        for s in range(nsteps + 2):
            fr = states.get(s)
            bk = states.get(s - 1)
            tl = states.get(s - 2)
            if s + 1 < nsteps:
                states[s + 1] = new_state(steps[s + 1])
                emit_rep(states[s + 1])
            if bk is not None:
                emit_xp(bk)
            for i in range(8):
                if fr is not None and i < 6:
                    emit_front_t(fr, i)
                    if i == 1:
                        emit_front_rsqrt(fr, 0)
                        emit_front_apply(fr, 0)
                    elif i == 5:
                        emit_front_rsqrt(fr, 1)
                        emit_front_apply(fr, 1)
                if bk is not None:
                    emit_ffn1_m(bk, i)
            if bk is not None:
                emit_ffn2_ln2(bk)
            if tl is not None:
                emit_ln2_store(tl)
                del states[s - 2]
            if s == 0 and fr is not None:
                # fill the unpipelined first-front latency so the PE p-state
                # survives into step 1
                for w in range(36):
                    wt = ps_m.tile([128, D], BF16, tag="m", name="wt")
                    nc.tensor.transpose(wt[:, 0:128], identb_sb[:],
                                        identb_sb[:])

        cstack.close()

    return nc


def kernel(H_genes, perturbation_indices, batch_assignment, batch_size,
           in_proj_w, in_proj_b, out_proj_w, out_proj_b,
           ffn_w1, ffn_b1, ffn_w2, ffn_b2,
           ln1_g, ln1_b, ln2_g, ln2_b):
    import ml_dtypes
    bf16 = ml_dtypes.bfloat16

    Hg = np.ascontiguousarray(np.asarray(H_genes, dtype=np.float32))
    pidx = np.asarray(perturbation_indices).astype(np.int64)
    ba = np.asarray(batch_assignment).astype(np.int64)
    Bs = int(np.asarray(batch_size))
    assert Bs == B, f"kernel hardcodes B=16, got {Bs}"
    assert Hg.shape == (N, D)

    Wq, Wk, Wv = [np.asarray(w, np.float32) for w in np.split(np.asarray(in_proj_w), 3, axis=0)]
    bq, bk, bv = [np.asarray(x, np.float32) for x in np.split(np.asarray(in_proj_b), 3, axis=0)]
    Wo = np.asarray(out_proj_w, np.float32)
    bo = np.asarray(out_proj_b, np.float32)
    W1 = np.asarray(ffn_w1, np.float32)
    b1 = np.asarray(ffn_b1, np.float32)
    W2 = np.asarray(ffn_w2, np.float32)
    b2 = np.asarray(ffn_b2, np.float32)
    g1 = np.asarray(ln1_g, np.float32)
    be1 = np.asarray(ln1_b, np.float32)
    g2 = np.asarray(ln2_g, np.float32)
    be2 = np.asarray(ln2_b, np.float32)

    counts = np.bincount(ba, minlength=B).astype(np.int64)
    has_any = counts > 0

    # host-side small projections: k and Wo-folded values
    Hp = Hg[pidx]                                   # [P, D]
    k = Hp @ Wk.T + bk[None, :]                     # [P, D]
    v = Hp @ Wv.T + bv[None, :]                     # [P, D]
    # vo[(h,p),:] = v[p, h-slice] @ Wo[:, h-slice].T  (full attn_out proj)
    voA = np.zeros((NGRP, 128, D), np.float32)
    for g in range(NGRP):
        for h in range(H):
            vh = v[g * GW : (g + 1) * GW, h * DH : (h + 1) * DH]   # [16, 32]
            voA[g, h * GW : (h + 1) * GW, :] = vh @ Wo[:, h * DH : (h + 1) * DH].T

    # per-head den stationaries: m01h[h][p, h*16+b] = 1{ba[p]==b}
    m01hA = np.zeros((H, 128, 128), np.float32)
    for h in range(H):
        for p in range(P):
            m01hA[h, p, h * GW + ba[p]] = 1.0

    # per-(batch, group) masked replication matrices:
    # rep[(h,b), (h,i)] = 1{ba[g*16+i] == b}
    blocks = {b: [] for b in range(B)}
    rep_mats = []
    for b in range(B):
        if counts[b] == 0:
            continue
        for g in range(NGRP):
            sel = ba[g * GW : (g + 1) * GW] == b
            if not sel.any():
                continue
            R = np.zeros((128, 128), np.float32)
            for h in range(H):
                for i in range(GW):
                    if sel[i]:
                        R[h * GW + b, h * GW + i] = 1.0
            blocks[b].append((g, len(rep_mats)))
            rep_mats.append(R)
    npair = len(rep_mats)

    # groups needed by the first two jobs get direct (overlapped) regroup
    jobs_order = [b for b in range(B) if counts[b] > 0]
    early_gs = [g for b in jobs_order[:2] for (g, _) in blocks[b]]
    n_early = (max(early_gs) + 1) if early_gs else 0

    # fold ln1 gain into FFN1 (exact): W1' = W1*g1, b1' = W1@b1_ln + b1
    W1f = W1 * g1[None, :]
    b1f = b1 + W1 @ be1

    Hg_pad = np.zeros((NPAD, D), np.float32)
    Hg_pad[:N] = Hg
    emptypA = np.tile((~has_any).astype(np.float32), H)[:, None]  # [(h,b),1]

    flags = (
        bool(np.any(bq != 0)), bool(np.any(b1f != 0)), bool(np.any(b2 != 0)),
        bool(np.any(g1 != 1)), bool(np.any(be1 != 0)),
        bool(np.any(g2 != 1)), bool(np.any(be2 != 0)),
    )
    use_bo = bool(np.any(bo != 0))

    nc = _build_program(counts, blocks, npair, n_early, flags)

    def tile128(a, inner):
        """[K*128, inner...] -> [128, K, inner...] partition-major."""
        a = np.ascontiguousarray(a)
        kdim = a.shape[0] // 128
        return np.ascontiguousarray(
            a.reshape(kdim, 128, *a.shape[1:]).transpose(
                1, 0, *range(2, a.ndim + 1)))

    common = {
        "kt": tile128(k.T, P).astype(np.float32),
        "wqt": tile128(Wq.T, D).astype(bf16),
        "bq_col": tile128(bq[:, None], 1),
        "vo": np.ascontiguousarray(voA.transpose(1, 0, 2)).astype(bf16),
        "m01h": np.ascontiguousarray(m01hA.transpose(1, 0, 2)).astype(bf16),
        "repm": (np.stack(rep_mats).transpose(1, 0, 2) if npair
                 else np.zeros((128, 1, 128), np.float32)).astype(bf16),
        "identb": np.eye(128, dtype=np.float32).astype(bf16),
        "emptyp": np.ascontiguousarray(emptypA),
        "w1t": tile128(W1f.T, F1).astype(bf16),
        "w2r": tile128(W2.T, D).astype(bf16),
        "b1_col": tile128(b1f[:, None], 1),
        "ln1c": tile128(np.stack([g1, be1], axis=1), 2),
        "gb_row": np.stack([b2, be1, g2, be2, g1, be1], axis=0),
    }
    in_maps = []
    for c in range(NCORES):
        sl = Hg_pad[c * NG : (c + 1) * NG]
        m = dict(common)
        hgb = sl + bo[None, :] if use_bo else sl
        m["hgb"] = tile128(hgb, D)
        m["hgt"] = tile128(np.ascontiguousarray(sl.T), NG).astype(bf16)
        in_maps.append(m)

    if os.environ.get("BASS_KERNEL_SIM"):
        from concourse import bass_interp
        # CoreSim lacks a Gelu LUT; shim exact (erf) gelu for local debugging.
        if not getattr(bass_interp.InstructionExecutor, "_gelu_patched", False):
            from scipy.special import erf
            _orig_act = bass_interp.InstructionExecutor.visit_InstActivation

            def _act(self, instruction, *, reg_snapshot=None):
                if instruction.func == mybir.ActivationFunctionType.Gelu:
                    instruction.func = mybir.ActivationFunctionType.Identity
                    try:
                        import concourse.bass_interp as bi
                        out_ap = instruction.outs[0]
                        r = _orig_act(self, instruction, reg_snapshot=reg_snapshot)
                        view = self.view_ap(out_ap, bi.Direction.READ, instruction,
                                            reg_snapshot=reg_snapshot)
                        x = view.astype(np.float64)
                        view[:] = (0.5 * x * (1.0 + erf(x / np.sqrt(2.0)))).astype(view.dtype)
                        return r
                    finally:
                        instruction.func = mybir.ActivationFunctionType.Gelu
                return _orig_act(self, instruction, reg_snapshot=reg_snapshot)

            bass_interp.InstructionExecutor.visit_InstActivation = _act
            bass_interp.InstructionExecutor._gelu_patched = True
        nsim = int(os.environ.get("BASS_KERNEL_SIM_CORES", "1"))
        simtrace = bool(os.environ.get("BASS_KERNEL_SIMTRACE"))
        sim = bass_interp.MultiCoreSim(nc, nsim, trace=simtrace)
        for c in range(nsim):
            for kk, vv in in_maps[c].items():
                sim.cores[c].tensor(kk)[:] = vv
        sim.simulate()
        print(f"SIM predicted time: {sim.cores[0].time} ns")
        full = np.zeros((B, NPAD, D), np.float32)
        for c in range(nsim):
            full[:, c * NG : (c + 1) * NG, :] = (
                np.array(sim.cores[c].mem_tensor("out")).reshape(B, NG, D))
        return full[:, :N, :]

    from concourse.bass_utils import run_bass_kernel_spmd
    _split_waits(nc)
    trace = bool(os.environ.get("BASS_KERNEL_TRACE"))
    res = run_bass_kernel_spmd(nc, in_maps, core_ids=list(range(NCORES)),
                               trace=trace)
    if trace and res.exec_time_ns is not None:
        print(f"HW exec time: {res.exec_time_ns} ns")
        if res.instructions_and_trace:
            print("trace:", res.instructions_and_trace[1])

    full = np.zeros((B, NPAD, D), np.float32)
    for c in range(NCORES):
        full[:, c * NG : (c + 1) * NG, :] = res.results[c]["out"]
    return full[:, :N, :]


# revision 32
# speedup vs baseline: 1.1175x; 1.1175x over previous
"""Trainium2 Bass kernel for nn_EquivariantPerturbationTransform.

Reference (N=6000 genes, D=256, H=8 heads, P=128 perturbations, B=16):
  q = H @ Wq.T ; k,v from gathered perturbation rows
  scores[h,n,p] shared across batches; per-batch mask over p (ragged)
  attn_out[b] = softmax-masked attention -> out proj (zeroed for empty b)
  x = LN1(H + attn_out); out = LN2(x + gelu(x@W1.T)@W2.T)

v3 strategy (sequence-parallel over 8 cores, 768 query rows/core):
  - Wo folded into values on the host (vo), so the attention context IS the
    projected attn_out (as v2).
  - Normalized attention weights Ehatall[(h,p16), n] = E/den computed ONCE
    for all batches (den from Et directly via per-head mask stationaries;
    replication matmul rep_all broadcasts each row's own batch denominator).
    Per (batch, group) the stationary is a row-masked copy (one DVE
    tensor_scalar with a 0/1 mask column); pure single-batch groups use
    Ehatall directly.
  - E head-layout -> block-layout regroup via a DRAM round-trip (8 writes +
    1 readback) instead of 64 SBUF-SBUF DMA triggers.
  - H residual and FFN residual moved off the PE onto Pool (tensor_tensor
    adds from PSUM); LN stats on DVE from bf16 SBUF tensors.
  - FFN2 computed in ROW layout (stationary = gelu-output slices, moving =
    W2 row-major k-tiles) so its output lands where LN2 needs it: no yT
    evac and no output transposes.
  - PE p-state: the tensor engine doubles to ~2.4GHz after ~3us of gap-free
    execution. Emission orders the PE queue [xpose | FFN1(m0-3) | ctx |
    FFN1(m4-7) | FFN2] per step with all dependencies scheduled to land
    before the PE reaches them, and PSUM pools sized so no matmul ever
    waits on an evacuation.
  - rsqrt for the LNs via quadratic seed + Newton step on DVE/Pool so the
    ACT table never leaves gelu after the phase-A Exp.
"""

import os
import sys

sys.path.insert(0, "/opt/trn_rl_repo")

import numpy as np

import concourse.bass as bass
from concourse import mybir
from concourse.tile import TileContext

F32 = mybir.dt.float32
F32R = mybir.dt.float32r
BF16 = mybir.dt.bfloat16
AF = mybir.ActivationFunctionType
ALU = mybir.AluOpType

N, D, H, P, B = 6000, 256, 8, 128, 16
DH = D // H  # 32
NCORES = 8
NPAD = 6144
NG = NPAD // NCORES  # 768 rows per core
NT = NG // 128       # 6 row tiles
EPS = 1e-5
GW = 16              # perturbation block width
NGRP = P // GW       # 8 blocks
F1 = 4 * D           # 1024


def _split_waits(nc, max_waits=1):
    """The neuronxcc/walrus build here rejects >1 sync-wait per instruction;
    hoist excess waits onto same-engine NoOps (semantically identical)."""
    n_split = 0
    for f in nc.m.functions:
        for bb in f.blocks:
            new_list = []
            for ins in bb.instructions:
                si = getattr(ins, "sync_info", None)
                if si is not None and si.on_wait and len(si.on_wait) > max_waits:
                    waits = list(si.on_wait)
                    excess, keep = waits[:-max_waits], waits[-max_waits:]
                    for i in range(0, len(excess), max_waits):
                        chunk = excess[i : i + max_waits]
                        nop = mybir.InstNoOp(name=f"{ins.name}-ws{i}", ins=[], outs=[])
                        nop.engine = ins.engine
                        nop.sync_info = mybir.SyncInfo(on_wait=chunk, on_update=[])
                        new_list.append(nop)
                        n_split += 1
                    si.on_wait = keep
                new_list.append(ins)
            bb.instructions = new_list
    return n_split


def _build_program(counts, blocks, npair, n_early, flags):
    """blocks[b] = list of (g, pair_idx) block descriptors; groups < n_early
    are regrouped with direct per-(g,h) DMAs, the rest via DRAM staging."""
    (use_bq, use_b1, use_b2, use_g1, use_b1ln, use_g2, use_b2ln) = flags
    nc = bass.Bass()

    # ---- DRAM parameters (already in on-chip [128, ...] layouts) ---------
    hgb = nc.declare_dram_parameter("hgb", [128, NT, D], F32, isOutput=False)
    hgt = nc.declare_dram_parameter("hgt", [128, 2, NG], BF16, isOutput=False)
    kt = nc.declare_dram_parameter("kt", [128, 2, P], F32R, isOutput=False)
    wqt = nc.declare_dram_parameter("wqt", [128, 2, D], BF16, isOutput=False)
    bq_col = nc.declare_dram_parameter("bq_col", [128, 2, 1], F32, isOutput=False)
    vo = nc.declare_dram_parameter("vo", [128, NGRP, D], BF16, isOutput=False)
    m01h = nc.declare_dram_parameter("m01h", [128, H, 128], BF16, isOutput=False)
    repm = nc.declare_dram_parameter("repm", [128, max(npair, 1), 128], BF16, isOutput=False)
    identb = nc.declare_dram_parameter("identb", [128, 128], BF16, isOutput=False)
    emptyp = nc.declare_dram_parameter("emptyp", [128, 1], F32, isOutput=False)
    w1t = nc.declare_dram_parameter("w1t", [128, 2, F1], BF16, isOutput=False)
    w2r = nc.declare_dram_parameter("w2r", [128, 8, D], BF16, isOutput=False)
    b1_col = nc.declare_dram_parameter("b1_col", [128, 8, 1], F32, isOutput=False)
    ln1c = nc.declare_dram_parameter("ln1c", [128, 2, 2], F32, isOutput=False)
    gb_row = nc.declare_dram_parameter("gb_row", [6, D], F32, isOutput=False)
    out = nc.declare_dram_parameter("out", [B, NG, D], F32, isOutput=True)

    s_attn = 1.0 / float(np.sqrt(DH))
    any_empty = any(int(c) == 0 for c in counts)
    jobs = [b for b in range(B) if int(counts[b]) > 0]

    with TileContext(nc) as tc, nc.allow_low_precision(
            reason="bf16 matmul inputs; tolerance budget is 2e-2 of max"):
        import contextlib
        import itertools

        _dma_engines = itertools.cycle([nc.sync, nc.gpsimd])

        def dma(out_ap, in_ap):
            next(_dma_engines).dma_start(out=out_ap, in_=in_ap)

        cstack = contextlib.ExitStack()
        consts = cstack.enter_context(tc.tile_pool(name="consts", bufs=1))
        dramp = cstack.enter_context(tc.tile_pool(name="dramp", bufs=1, space="DRAM"))

        # ---- persistent constants (few, bundled DMAs; critical first) ----
        identb_sb = consts.tile([128, 128], BF16, tag="identb", name="identb_sb")
        dma(identb_sb[:], identb[:, :])
        hgt_sb = consts.tile([128, 2, NG], BF16, tag="hgt", name="hgt_sb")
        dma(hgt_sb[:], hgt[:, :, :])
        wq_sb = consts.tile([128, 2, D], BF16, tag="wq", name="wq_sb")
        dma(wq_sb[:], wqt[:, :, :])
        kt_sb = consts.tile([128, 2, P], F32R, tag="kt", name="kt_sb")
        dma(kt_sb[:], kt[:, :, :])
        hgb_sb = consts.tile([128, NT, D], F32, tag="hgb", name="hgb_sb")
        dma(hgb_sb[:], hgb[:, :, :])
        w1_sb = consts.tile([128, 2, F1], BF16, tag="w1", name="w1_sb")
        dma(w1_sb[:], w1t[:, :, :])
        w2_sb = consts.tile([128, 8, D], BF16, tag="w2", name="w2_sb")
        dma(w2_sb[:], w2r[:, :, :])
        vo_sb = consts.tile([128, NGRP, D], BF16, tag="vo", name="vo_sb")
        dma(vo_sb[:], vo[:, :, :])
        m01_sb = consts.tile([128, H, 128], BF16, tag="m01", name="m01_sb")
        dma(m01_sb[:], m01h[:, :, :])
        rep_sb = consts.tile([128, max(npair, 1), 128], BF16, tag="rep", name="rep_sb")
        dma(rep_sb[:], repm[:, :, :])
        empty_sb = consts.tile([128, 1], F32, tag="empty", name="empty_sb")
        dma(empty_sb[:], emptyp[:, :])
        b1_sb = consts.tile([128, 8, 1], F32, tag="b1c", name="b1_sb")
        dma(b1_sb[:], b1_col[:, :, :])
        bq_sb = None
        if use_bq:
            bq_sb = consts.tile([128, 2, 1], F32, tag="bqc", name="bq_sb")
            dma(bq_sb[:], bq_col[:, :, :])
        ln1_sb = None
        if use_g1 or use_b1ln:
            ln1_sb = consts.tile([128, 2, 2], F32, tag="ln1c", name="ln1_sb")
            dma(ln1_sb[:], ln1c[:, :, :])
        gbr_sb = None
        if use_g2 or use_b2ln or use_b2 or use_g1 or use_b1ln:
            gbr_sb = consts.tile([128, 6, D], F32, tag="gbr", name="gbr_sb")
            nc.gpsimd.dma_start(out=gbr_sb[:], in_=gb_row[:, :].to_broadcast((128, 6, D)))

        # persistent activations
        qT_sb = consts.tile([128, 2, NG], F32R, tag="qT", name="qT_sb")
        Et = consts.tile([128, H, NG], BF16, tag="Et", name="Et")
        Eg = consts.tile([128, NGRP, NG], BF16, tag="Eg", name="Eg")
        den_sb = consts.tile([128, NG], BF16, tag="den", name="den_sb")
        denf_sb = consts.tile([128, NG], F32, tag="denf", name="denf_sb")
        denr_sb = consts.tile([128, NG], BF16, tag="denr", name="denr_sb")
        n_stage = NGRP - n_early
        stg = dramp.tile([max(n_stage, 1), H, GW, NG], BF16, tag="stg", name="stg")

        NCH = 2
        CH = NG // NCH  # 384

        # ================= Phase A: shared projections ==================
        with tc.tile_pool(name="psA", bufs=1, space="PSUM") as psA:
            # PE warmup: ramp the tensor-engine p-state while DMAs land
            wtile = psA.tile([128, NG], BF16, tag="wt", name="wtile")
            for w in range(12):
                nc.tensor.transpose(wtile[:, 0:128], identb_sb[:], identb_sb[:])

            # qT [D, NG] = Wq^T Hg^T  (PSUM writes split at the bank edge)
            for m in range(2):
                ps = psA.tile([128, NG], F32, tag=f"qs{m % 2}", name="ps_q")
                for kk in range(2):
                    for lo, hi in ((0, 512), (512, NG)):
                        nc.tensor.matmul(
                            ps[:, lo:hi], wq_sb[:, kk, m * 128 : (m + 1) * 128],
                            hgt_sb[:, kk, lo:hi], start=(kk == 0), stop=(kk == 1))
                if use_bq:
                    nc.scalar.activation(qT_sb[:, m, :], ps[:], AF.Identity,
                                         bias=bq_sb[:, m, 0:1])
                else:
                    nc.scalar.activation(qT_sb[:, m, :], ps[:], AF.Copy)

            # Et[p, h, n] = exp(s * k_h q_h^T) per head, with the den
            # accumulation matmuls and the early-group regroup DMAs
            # interleaved so the PE never drains during the exp chain.
            psd = [psA.tile([128, CH], F32, tag=f"dn{c}", name=f"psd{c}")
                   for c in range(NCH)]

            def den_mm(h):
                for c in range(NCH):
                    nc.tensor.matmul(
                        psd[c][:], m01_sb[:, h, :],
                        Et[:, h, c * CH : (c + 1) * CH],
                        start=(h == 0), stop=(h == H - 1))

            for h in range(H):
                ps = psA.tile([128, NG], F32, tag=f"qs{h % 2}", name="ps_s")
                for lo, hi in ((0, 512), (512, NG)):
                    nc.tensor.matmul(
                        ps[:, lo:hi],
                        kt_sb[(h % 4) * DH : (h % 4 + 1) * DH, h // 4, :],
                        qT_sb[(h % 4) * DH : (h % 4 + 1) * DH, h // 4, lo:hi],
                        start=True, stop=True,
                        tile_position=((h % 4) * DH, 0))
                nc.scalar.activation(Et[:, h, :], ps[:], AF.Exp, scale=s_attn)
                if h >= 1:
                    den_mm(h - 1)
                for w in range(6):
                    nc.tensor.transpose(wtile[:, 0:128], identb_sb[:],
                                        identb_sb[:])
                for g in range(n_early):
                    dma(Eg[h * GW : (h + 1) * GW, g, :],
                        Et[g * GW : (g + 1) * GW, h, :])
            den_mm(H - 1)

            # den -> reciprocal (fast-approx) -> bf16
            for c in range(NCH):
                nc.vector.tensor_scalar(
                    out=den_sb[:, c * CH : (c + 1) * CH], in0=psd[c][:],
                    scalar1=empty_sb[:, 0:1], scalar2=None, op0=ALU.add)
                nc.vector.reciprocal(
                    out=denr_sb[:, c * CH : (c + 1) * CH],
                    in_=den_sb[:, c * CH : (c + 1) * CH])

            # hold the PE p-state while the reciprocal chain runs
            for w in range(44):
                nc.tensor.transpose(wtile[:, 0:128], identb_sb[:], identb_sb[:])

            # late groups: Et -> DRAM staging -> Eg block layout
            for si in range(n_stage):
                g = n_early + si
                dma(stg[si].rearrange("h i n -> i h n"),
                    Et[g * GW : (g + 1) * GW, :, :])
            if n_stage:
                nc.sync.dma_start(
                    out=Eg[:, n_early:NGRP, :],
                    in_=stg[:].rearrange("g h i n -> (h i) g n"))

        # ================= Phase B: per-batch pipeline ==================
        work = cstack.enter_context(tc.tile_pool(name="work", bufs=2))
        ehpool = cstack.enter_context(tc.tile_pool(name="ehp", bufs=6))
        ps_f1 = cstack.enter_context(tc.tile_pool(name="ps_f1", bufs=2, space="PSUM"))
        ps_m = cstack.enter_context(tc.tile_pool(name="ps_m", bufs=2, space="PSUM"))
        ps_y = cstack.enter_context(tc.tile_pool(name="ps_y", bufs=2, space="PSUM"))

        RC2, RC1, RC0 = 0.29333931447269, -1.1711876763158582, 1.8939170369253155

        def rsqrt_group(vv_ap, mu_ap, sc_ap, ng_ap, sA, sB, eng=None):
            """sc = rsqrt(vv), ng = -mu*sc. Quadratic seed + one Newton step.
            All-DVE by default (cross-engine hops cost semaphore latency);
            with eng=gpsimd the tensor_tensor ops go to Pool (immediate
            tensor_scalar is DVE-only in this toolchain) for chains with
            timing slack."""
            e = eng if eng is not None else nc.vector
            nc.vector.tensor_scalar(out=sA, in0=vv_ap, scalar1=RC2,
                                    scalar2=RC1, op0=ALU.mult, op1=ALU.add)
            e.tensor_tensor(out=sA, in0=sA, in1=vv_ap, op=ALU.mult)
            nc.vector.tensor_scalar(out=sA, in0=sA, scalar1=RC0,
                                    scalar2=None, op0=ALU.add)
            e.tensor_tensor(out=sB, in0=sA, in1=sA, op=ALU.mult)
            e.tensor_tensor(out=sB, in0=sB, in1=vv_ap, op=ALU.mult)
            nc.vector.tensor_scalar(out=sB, in0=sB, scalar1=-0.5,
                                    scalar2=1.5, op0=ALU.mult, op1=ALU.add)
            e.tensor_tensor(out=sc_ap, in0=sA, in1=sB, op=ALU.mult)
            nc.vector.scalar_tensor_tensor(
                out=ng_ap, in0=mu_ap, scalar=-1.0, in1=sc_ap,
                op0=ALU.mult, op1=ALU.mult)

        def new_state(b):
            return {
                "b": b,
                "mvb": work.tile([128, NT, 2], F32, tag="mvb", name="mvb"),
                "sc1": work.tile([128, NT], F32, tag="sc1", name="sc1"),
                "ng1": work.tile([128, NT], F32, tag="ng1", name="ng1"),
                "stats": work.tile([128, 2, 6], F32, tag="stats", name="stats"),
                "xpre": work.tile([128, NT, D], BF16, tag="xpre", name="xpre"),
                "x_row": work.tile([128, NT, D], BF16, tag="x_row", name="x_row",
                                   bufs=3),
                "xT": work.tile([128, 2, NG], BF16, tag="xT", name="xT"),
                "sA": work.tile([128, 6], F32, tag="sA", name="sA"),
                "sB": work.tile([128, 6], F32, tag="sB", name="sB"),
                "eh": [],
            }

        def emit_rep(st):
            """Per-(batch,group) masked replication matmul (PE) + normalize
            multiply (DVE): eh = E_g * masked-replicated reciprocal."""
            b = st["b"]
            if b is None:
                return
            for g, ri in blocks[b]:
                tl = ehpool.tile([128, NG], BF16, tag="eh", name="eh")
                for c in range(NCH):
                    psr = ps_m.tile([128, CH], F32, tag="m", name="psr")
                    nc.tensor.matmul(
                        psr[:], rep_sb[:, ri, :],
                        denr_sb[:, c * CH : (c + 1) * CH],
                        start=True, stop=True)
                    nc.vector.tensor_mul(
                        tl[:, c * CH : (c + 1) * CH],
                        Eg[:, g, c * CH : (c + 1) * CH], psr[:])
                st["eh"].append((g, tl[:]))

        def emit_xp(st):
            """x transposes (PE) + xT evac (split ACT/DVE), per tile."""
            for t in range(NT):
                psx = ps_m.tile([128, D], BF16, tag="m", name="psx")
                for k in range(2):
                    nc.tensor.transpose(
                        psx[:, k * 128 : (k + 1) * 128],
                        st["x_row"][:, t, k * 128 : (k + 1) * 128],
                        identb_sb[:])
                if t < 4:
                    nc.scalar.activation(
                        st["xT"][:, :, t * 128 : (t + 1) * 128],
                        psx[:].rearrange("p (k n) -> p k n", k=2), AF.Copy)
                else:
                    nc.vector.tensor_copy(
                        out=st["xT"][:, :, t * 128 : (t + 1) * 128],
                        in_=psx[:].rearrange("p (k n) -> p k n", k=2))

        def emit_ffn1_m(st, m):
            """One FFN1 m-tile: PE matmuls + ACT gelu."""
            if m == 0:
                st["h1g"] = work.tile([128, 8, NG], BF16, tag="h1g", name="h1g")
            xT = st["xT"]
            ps = ps_f1.tile([128, NG], F32, tag="f1", name="f1")
            for kk in range(2):
                nc.tensor.matmul(ps[:, 0:512], w1_sb[:, kk, m * 128 : (m + 1) * 128],
                                 xT[:, kk, 0:512],
                                 start=(kk == 0), stop=(kk == 1))
                nc.tensor.matmul(ps[:, 512:NG], w1_sb[:, kk, m * 128 : (m + 1) * 128],
                                 xT[:, kk, 512:NG],
                                 start=(kk == 0), stop=(kk == 1))
            if use_b1:
                nc.scalar.activation(st["h1g"][:, m, :], ps[:], AF.Gelu,
                                     bias=b1_sb[:, m, 0:1])
            else:
                nc.scalar.activation(st["h1g"][:, m, :], ps[:], AF.Gelu)

        def emit_front_t(st, t):
            """One front tile: ctx matmuls (PE) + H residual evac (DVE);
            LN1 stats (DVE) in tile pairs."""
            b = st["b"]
            if b is not None:
                psa = ps_m.tile([128, D], F32, tag="m", name="ao")
                nb = len(st["eh"])
                for i, (g, eh_ap) in enumerate(st["eh"]):
                    nc.tensor.matmul(
                        psa[:], eh_ap[:, t * 128 : (t + 1) * 128],
                        vo_sb[:, g, :], start=(i == 0), stop=(i == nb - 1))
                nc.vector.scalar_tensor_tensor(
                    out=st["xpre"][:, t, :], in0=psa[:], scalar=1.0,
                    in1=hgb_sb[:, t, :], op0=ALU.mult, op1=ALU.add)
            src_t = st["xpre"] if b is not None else hgb_sb
            nc.vector.bn_stats(out=st["stats"][:, 0, :], in_=src_t[:, t, :])
            nc.vector.bn_aggr(out=st["mvb"][:, t, :], in_=st["stats"][:, 0, :])

        def emit_front_rsqrt(st, half):
            lo, hi = (0, 2) if half == 0 else (2, 6)
            mvb, sc1, ng1 = st["mvb"], st["sc1"], st["ng1"]
            rsqrt_group(mvb[:, lo:hi, 1], mvb[:, lo:hi, 0],
                        sc1[:, lo:hi], ng1[:, lo:hi],
                        st["sA"][:, lo:hi], st["sB"][:, lo:hi])

        def emit_front_apply(st, half):
            """LN1 applies (Pool tensor_scalar, per-partition AP scalars)."""
            b = st["b"]
            sc1, ng1 = st["sc1"], st["ng1"]
            src = st["xpre"] if b is not None else hgb_sb
            for t in (range(0, 2) if half == 0 else range(2, 6)):
                nc.gpsimd.tensor_scalar(
                    out=st["x_row"][:, t, :], in0=src[:, t, :],
                    scalar1=sc1[:, t : t + 1], scalar2=ng1[:, t : t + 1],
                    op0=ALU.mult, op1=ALU.add)

        def emit_ffn2_ln2(st):
            """FFN2 in row layout (PE, t-outer k-inner) + y residual evac
            (DVE) + LN2 stats (DVE)."""
            st["ypre"] = work.tile([128, NT, D], BF16, tag="ypre", name="ypre")
            st["mv2"] = work.tile([128, NT, 2], F32, tag="mv2", name="mv2")
            st["st2"] = work.tile([128, 2, 6], F32, tag="st2", name="st2")
            h1g = st["h1g"]
            xres = st["x_row"]
            if use_g1 or use_b1ln:
                # residual needs the true x = x_norm*g1 + be1 (g1 folded into
                # W1 elsewhere); rare flag path
                xres = work.tile([128, NT, D], BF16, tag="xres", name="xres")
                for t in range(NT):
                    nc.vector.tensor_mul(xres[:, t, :], st["x_row"][:, t, :],
                                         gbr_sb[:, 4, :])
                    nc.vector.tensor_add(xres[:, t, :], xres[:, t, :],
                                         gbr_sb[:, 5, :])
            for t in range(NT):
                psy = ps_y.tile([128, D], F32, tag="y", name="psy")
                for kk in range(8):
                    nc.tensor.matmul(
                        psy[:], h1g[:, kk, t * 128 : (t + 1) * 128],
                        w2_sb[:, kk, :], start=(kk == 0), stop=(kk == 7))
                if use_b2:
                    nc.vector.tensor_add(psy[:], psy[:], gbr_sb[:, 0, :])
                nc.vector.scalar_tensor_tensor(
                    out=st["ypre"][:, t, :], in0=psy[:], scalar=1.0,
                    in1=xres[:, t, :], op0=ALU.mult, op1=ALU.add)
                nc.vector.bn_stats(out=st["st2"][:, 0, :],
                                   in_=st["ypre"][:, t, :])
                nc.vector.bn_aggr(out=st["mv2"][:, t, :],
                                  in_=st["st2"][:, 0, :])

        def emit_ln2_store(st):
            """LN2 rsqrt (DVE+Pool) + apply (Pool) + store."""
            b = st["b"]
            sc2 = work.tile([128, NT], F32, tag="sc2", name="sc2")
            ng2 = work.tile([128, NT], F32, tag="ng2", name="ng2")
            sA2 = work.tile([128, 6], F32, tag="sA2", name="sA2")
            sB2 = work.tile([128, 6], F32, tag="sB2", name="sB2")
            orow = work.tile([128, NT, D], F32, tag="orow", name="orow")
            ypre, mv2 = st["ypre"], st["mv2"]
            rsqrt_group(mv2[:, :, 1], mv2[:, :, 0], sc2[:, :], ng2[:, :],
                        sA2[:], sB2[:])
            for t in range(NT):
                nc.gpsimd.tensor_scalar(
                    out=orow[:, t, :], in0=ypre[:, t, :],
                    scalar1=sc2[:, t : t + 1], scalar2=ng2[:, t : t + 1],
                    op0=ALU.mult, op1=ALU.add)
                if use_g2:
                    nc.vector.tensor_mul(orow[:, t, :], orow[:, t, :],
                                         gbr_sb[:, 2, :])
                if use_b2ln:
                    nc.vector.tensor_add(orow[:, t, :], orow[:, t, :],
                                         gbr_sb[:, 3, :])
            if b is not None:
                dma(out[b].rearrange("(t p) d -> p t d", p=128), orow[:])
            else:
                for be in range(B):
                    if int(counts[be]) == 0:
                        dma(out[be].rearrange("(t p) d -> p t d", p=128),
                            orow[:])

        # ---- software-pipelined emission --------------------------------
        # PE queue per step: xp(bk) | FFN1-m(bk) interleaved with ctx-t(fr)
        # | FFN2(bk).  ACT queue: gelus(bk) | LN1 applies(fr).  The m/t
        # interleave spaces PSUM-slot reuse past the Pool/DVE drain latency
        # so the PE stream never stalls.
        steps = ([None] if any_empty else []) + jobs
        nsteps = len(steps)
        states = {}
        states[0] = new_state(steps[0])
        emit_rep(states[0])
        for s in range(nsteps + 2):
            fr = states.get(s)
            bk = states.get(s - 1)
            tl = states.get(s - 2)
            if bk is not None:
                emit_xp(bk)
            for i in range(8):
                if fr is not None and i < 6:
                    emit_front_t(fr, i)
                    if i == 1:
                        emit_front_rsqrt(fr, 0)
                        emit_front_apply(fr, 0)
                    elif i == 5:
                        emit_front_rsqrt(fr, 1)
                        emit_front_apply(fr, 1)
                if bk is not None:
                    emit_ffn1_m(bk, i)
            if s + 1 < nsteps:
                states[s + 1] = new_state(steps[s + 1])
                emit_rep(states[s + 1])
            if bk is not None:
                emit_ffn2_ln2(bk)
            if tl is not None:
                emit_ln2_store(tl)
                del states[s - 2]
            if s == 0 and fr is not None:
                # fill the unpipelined first-front latency so the PE p-state
                # survives into step 1
                for w in range(36):
                    wt = ps_m.tile([128, D], BF16, tag="m", name="wt")
                    nc.tensor.transpose(wt[:, 0:128], identb_sb[:],
                                        identb_sb[:])

        cstack.close()

    return nc


def kernel(H_genes, perturbation_indices, batch_assignment, batch_size,
           in_proj_w, in_proj_b, out_proj_w, out_proj_b,
           ffn_w1, ffn_b1, ffn_w2, ffn_b2,
           ln1_g, ln1_b, ln2_g, ln2_b):
    import ml_dtypes
    bf16 = ml_dtypes.bfloat16

    Hg = np.ascontiguousarray(np.asarray(H_genes, dtype=np.float32))
    pidx = np.asarray(perturbation_indices).astype(np.int64)
    ba = np.asarray(batch_assignment).astype(np.int64)
    Bs = int(np.asarray(batch_size))
    assert Bs == B, f"kernel hardcodes B=16, got {Bs}"
    assert Hg.shape == (N, D)

    Wq, Wk, Wv = [np.asarray(w, np.float32) for w in np.split(np.asarray(in_proj_w), 3, axis=0)]
    bq, bk, bv = [np.asarray(x, np.float32) for x in np.split(np.asarray(in_proj_b), 3, axis=0)]
    Wo = np.asarray(out_proj_w, np.float32)
    bo = np.asarray(out_proj_b, np.float32)
    W1 = np.asarray(ffn_w1, np.float32)
    b1 = np.asarray(ffn_b1, np.float32)
    W2 = np.asarray(ffn_w2, np.float32)
    b2 = np.asarray(ffn_b2, np.float32)
    g1 = np.asarray(ln1_g, np.float32)
    be1 = np.asarray(ln1_b, np.float32)
    g2 = np.asarray(ln2_g, np.float32)
    be2 = np.asarray(ln2_b, np.float32)

    counts = np.bincount(ba, minlength=B).astype(np.int64)
    has_any = counts > 0

    # host-side small projections: k and Wo-folded values
    Hp = Hg[pidx]                                   # [P, D]
    k = Hp @ Wk.T + bk[None, :]                     # [P, D]
    v = Hp @ Wv.T + bv[None, :]                     # [P, D]
    # vo[(h,p),:] = v[p, h-slice] @ Wo[:, h-slice].T  (full attn_out proj)
    voA = np.zeros((NGRP, 128, D), np.float32)
    for g in range(NGRP):
        for h in range(H):
            vh = v[g * GW : (g + 1) * GW, h * DH : (h + 1) * DH]   # [16, 32]
            voA[g, h * GW : (h + 1) * GW, :] = vh @ Wo[:, h * DH : (h + 1) * DH].T

    # per-head den stationaries: m01h[h][p, h*16+b] = 1{ba[p]==b}
    m01hA = np.zeros((H, 128, 128), np.float32)
    for h in range(H):
        for p in range(P):
            m01hA[h, p, h * GW + ba[p]] = 1.0

    # per-(batch, group) masked replication matrices:
    # rep[(h,b), (h,i)] = 1{ba[g*16+i] == b}
    blocks = {b: [] for b in range(B)}
    rep_mats = []
    for b in range(B):
        if counts[b] == 0:
            continue
        for g in range(NGRP):
            sel = ba[g * GW : (g + 1) * GW] == b
            if not sel.any():
                continue
            R = np.zeros((128, 128), np.float32)
            for h in range(H):
                for i in range(GW):
                    if sel[i]:
                        R[h * GW + b, h * GW + i] = 1.0
            blocks[b].append((g, len(rep_mats)))
            rep_mats.append(R)
    npair = len(rep_mats)

    # groups needed by the first two jobs get direct (overlapped) regroup
    jobs_order = [b for b in range(B) if counts[b] > 0]
    early_gs = [g for b in jobs_order[:2] for (g, _) in blocks[b]]
    n_early = (max(early_gs) + 1) if early_gs else 0

    # fold ln1 gain into FFN1 (exact): W1' = W1*g1, b1' = W1@b1_ln + b1
    W1f = W1 * g1[None, :]
    b1f = b1 + W1 @ be1

    Hg_pad = np.zeros((NPAD, D), np.float32)
    Hg_pad[:N] = Hg
    emptypA = np.tile((~has_any).astype(np.float32), H)[:, None]  # [(h,b),1]

    flags = (
        bool(np.any(bq != 0)), bool(np.any(b1f != 0)), bool(np.any(b2 != 0)),
        bool(np.any(g1 != 1)), bool(np.any(be1 != 0)),
        bool(np.any(g2 != 1)), bool(np.any(be2 != 0)),
    )
    use_bo = bool(np.any(bo != 0))

    nc = _build_program(counts, blocks, npair, n_early, flags)

    def tile128(a, inner):
        """[K*128, inner...] -> [128, K, inner...] partition-major."""
        a = np.ascontiguousarray(a)
        kdim = a.shape[0] // 128
        return np.ascontiguousarray(
            a.reshape(kdim, 128, *a.shape[1:]).transpose(
                1, 0, *range(2, a.ndim + 1)))

    common = {
        "kt": tile128(k.T, P).astype(np.float32),
        "wqt": tile128(Wq.T, D).astype(bf16),
        "bq_col": tile128(bq[:, None], 1),
        "vo": np.ascontiguousarray(voA.transpose(1, 0, 2)).astype(bf16),
        "m01h": np.ascontiguousarray(m01hA.transpose(1, 0, 2)).astype(bf16),
        "repm": (np.stack(rep_mats).transpose(1, 0, 2) if npair
                 else np.zeros((128, 1, 128), np.float32)).astype(bf16),
        "identb": np.eye(128, dtype=np.float32).astype(bf16),
        "emptyp": np.ascontiguousarray(emptypA),
        "w1t": tile128(W1f.T, F1).astype(bf16),
        "w2r": tile128(W2.T, D).astype(bf16),
        "b1_col": tile128(b1f[:, None], 1),
        "ln1c": tile128(np.stack([g1, be1], axis=1), 2),
        "gb_row": np.stack([b2, be1, g2, be2, g1, be1], axis=0),
    }
    in_maps = []
    for c in range(NCORES):
        sl = Hg_pad[c * NG : (c + 1) * NG]
        m = dict(common)
        hgb = sl + bo[None, :] if use_bo else sl
        m["hgb"] = tile128(hgb, D)
        m["hgt"] = tile128(np.ascontiguousarray(sl.T), NG).astype(bf16)
        in_maps.append(m)

    if os.environ.get("BASS_KERNEL_SIM"):
        from concourse import bass_interp
        # CoreSim lacks a Gelu LUT; shim exact (erf) gelu for local debugging.
        if not getattr(bass_interp.InstructionExecutor, "_gelu_patched", False):
            from scipy.special import erf
            _orig_act = bass_interp.InstructionExecutor.visit_InstActivation

            def _act(self, instruction, *, reg_snapshot=None):
                if instruction.func == mybir.ActivationFunctionType.Gelu:
                    instruction.func = mybir.ActivationFunctionType.Identity
                    try:
                        import concourse.bass_interp as bi
                        out_ap = instruction.outs[0]
                        r = _orig_act(self, instruction, reg_snapshot=reg_snapshot)
                        view = self.view_ap(out_ap, bi.Direction.READ, instruction,
                                            reg_snapshot=reg_snapshot)
                        x = view.astype(np.float64)
                        view[:] = (0.5 * x * (1.0 + erf(x / np.sqrt(2.0)))).astype(view.dtype)
                        return r
                    finally:
                        instruction.func = mybir.ActivationFunctionType.Gelu
                return _orig_act(self, instruction, reg_snapshot=reg_snapshot)

            bass_interp.InstructionExecutor.visit_InstActivation = _act
            bass_interp.InstructionExecutor._gelu_patched = True
        nsim = int(os.environ.get("BASS_KERNEL_SIM_CORES", "1"))
        simtrace = bool(os.environ.get("BASS_KERNEL_SIMTRACE"))
        sim = bass_interp.MultiCoreSim(nc, nsim, trace=simtrace)
        for c in range(nsim):
            for kk, vv in in_maps[c].items():
                sim.cores[c].tensor(kk)[:] = vv
        sim.simulate()
        print(f"SIM predicted time: {sim.cores[0].time} ns")
        full = np.zeros((B, NPAD, D), np.float32)
        for c in range(nsim):
            full[:, c * NG : (c + 1) * NG, :] = (
                np.array(sim.cores[c].mem_tensor("out")).reshape(B, NG, D))
        return full[:, :N, :]

    from concourse.bass_utils import run_bass_kernel_spmd
    _split_waits(nc)
    trace = bool(os.environ.get("BASS_KERNEL_TRACE"))
    res = run_bass_kernel_spmd(nc, in_maps, core_ids=list(range(NCORES)),
                               trace=trace)
    if trace and res.exec_time_ns is not None:
        print(f"HW exec time: {res.exec_time_ns} ns")
        if res.instructions_and_trace:
            print("trace:", res.instructions_and_trace[1])

    full = np.zeros((B, NPAD, D), np.float32)
    for c in range(NCORES):
        full[:, c * NG : (c + 1) * NG, :] = res.results[c]["out"]
    return full[:, :N, :]


# revision 34
# speedup vs baseline: 1.1525x; 1.0313x over previous
"""Trainium2 Bass kernel for nn_EquivariantPerturbationTransform.

Reference (N=6000 genes, D=256, H=8 heads, P=128 perturbations, B=16):
  q = H @ Wq.T ; k,v from gathered perturbation rows
  scores[h,n,p] shared across batches; per-batch mask over p (ragged)
  attn_out[b] = softmax-masked attention -> out proj (zeroed for empty b)
  x = LN1(H + attn_out); out = LN2(x + gelu(x@W1.T)@W2.T)

v3 strategy (sequence-parallel over 8 cores, 768 query rows/core):
  - Wo folded into values on the host (vo), so the attention context IS the
    projected attn_out (as v2).
  - Normalized attention weights Ehatall[(h,p16), n] = E/den computed ONCE
    for all batches (den from Et directly via per-head mask stationaries;
    replication matmul rep_all broadcasts each row's own batch denominator).
    Per (batch, group) the stationary is a row-masked copy (one DVE
    tensor_scalar with a 0/1 mask column); pure single-batch groups use
    Ehatall directly.
  - E head-layout -> block-layout regroup via a DRAM round-trip (8 writes +
    1 readback) instead of 64 SBUF-SBUF DMA triggers.
  - H residual and FFN residual moved off the PE onto Pool (tensor_tensor
    adds from PSUM); LN stats on DVE from bf16 SBUF tensors.
  - FFN2 computed in ROW layout (stationary = gelu-output slices, moving =
    W2 row-major k-tiles) so its output lands where LN2 needs it: no yT
    evac and no output transposes.
  - PE p-state: the tensor engine doubles to ~2.4GHz after ~3us of gap-free
    execution. Emission orders the PE queue [xpose | FFN1(m0-3) | ctx |
    FFN1(m4-7) | FFN2] per step with all dependencies scheduled to land
    before the PE reaches them, and PSUM pools sized so no matmul ever
    waits on an evacuation.
  - rsqrt for the LNs via quadratic seed + Newton step on DVE/Pool so the
    ACT table never leaves gelu after the phase-A Exp.
"""

import os
import sys

sys.path.insert(0, "/opt/trn_rl_repo")

import numpy as np

import concourse.bass as bass
from concourse import mybir
from concourse.tile import TileContext

F32 = mybir.dt.float32
F32R = mybir.dt.float32r
BF16 = mybir.dt.bfloat16
AF = mybir.ActivationFunctionType
ALU = mybir.AluOpType

N, D, H, P, B = 6000, 256, 8, 128, 16
DH = D // H  # 32
NCORES = 8
NPAD = 6144
NG = NPAD // NCORES  # 768 rows per core
NT = NG // 128       # 6 row tiles
EPS = 1e-5
GW = 16              # perturbation block width
NGRP = P // GW       # 8 blocks
F1 = 4 * D           # 1024


def _split_waits(nc, max_waits=1):
    """The neuronxcc/walrus build here rejects >1 sync-wait per instruction;
    hoist excess waits onto same-engine NoOps (semantically identical)."""
    n_split = 0
    for f in nc.m.functions:
        for bb in f.blocks:
            new_list = []
            for ins in bb.instructions:
                si = getattr(ins, "sync_info", None)
                if si is not None and si.on_wait and len(si.on_wait) > max_waits:
                    waits = list(si.on_wait)
                    excess, keep = waits[:-max_waits], waits[-max_waits:]
                    for i in range(0, len(excess), max_waits):
                        chunk = excess[i : i + max_waits]
                        nop = mybir.InstNoOp(name=f"{ins.name}-ws{i}", ins=[], outs=[])
                        nop.engine = ins.engine
                        nop.sync_info = mybir.SyncInfo(on_wait=chunk, on_update=[])
                        new_list.append(nop)
                        n_split += 1
                    si.on_wait = keep
                new_list.append(ins)
            bb.instructions = new_list
    return n_split


def _build_program(counts, blocks, npair, n_early, flags):
    """blocks[b] = list of (g, pair_idx) block descriptors; groups < n_early
    are regrouped with direct per-(g,h) DMAs, the rest via DRAM staging."""
    (use_bq, use_b1, use_b2, use_g1, use_b1ln, use_g2, use_b2ln) = flags
    nc = bass.Bass()

    # ---- DRAM parameters (already in on-chip [128, ...] layouts) ---------
    hgb = nc.declare_dram_parameter("hgb", [128, NT, D], F32, isOutput=False)
    hgt = nc.declare_dram_parameter("hgt", [128, 2, NG], BF16, isOutput=False)
    kt = nc.declare_dram_parameter("kt", [128, 2, P], F32R, isOutput=False)
    wqt = nc.declare_dram_parameter("wqt", [128, 2, D], BF16, isOutput=False)
    bq_col = nc.declare_dram_parameter("bq_col", [128, 2, 1], F32, isOutput=False)
    vo = nc.declare_dram_parameter("vo", [128, NGRP, D], BF16, isOutput=False)
    m01h = nc.declare_dram_parameter("m01h", [128, H, 128], BF16, isOutput=False)
    repm = nc.declare_dram_parameter("repm", [128, max(npair, 1), 128], BF16, isOutput=False)
    identb = nc.declare_dram_parameter("identb", [128, 128], BF16, isOutput=False)
    emptyp = nc.declare_dram_parameter("emptyp", [128, 1], F32, isOutput=False)
    w1t = nc.declare_dram_parameter("w1t", [128, 2, F1], BF16, isOutput=False)
    w2r = nc.declare_dram_parameter("w2r", [128, 8, D], BF16, isOutput=False)
    b1_col = nc.declare_dram_parameter("b1_col", [128, 8, 1], F32, isOutput=False)
    ln1c = nc.declare_dram_parameter("ln1c", [128, 2, 2], F32, isOutput=False)
    gb_row = nc.declare_dram_parameter("gb_row", [6, D], F32, isOutput=False)
    out = nc.declare_dram_parameter("out", [B, NG, D], F32, isOutput=True)

    s_attn = 1.0 / float(np.sqrt(DH))
    any_empty = any(int(c) == 0 for c in counts)
    jobs = [b for b in range(B) if int(counts[b]) > 0]

    with TileContext(nc) as tc, nc.allow_low_precision(
            reason="bf16 matmul inputs; tolerance budget is 2e-2 of max"):
        import contextlib
        import itertools

        _dma_engines = itertools.cycle([nc.sync, nc.gpsimd])

        def dma(out_ap, in_ap):
            next(_dma_engines).dma_start(out=out_ap, in_=in_ap)

        cstack = contextlib.ExitStack()
        consts = cstack.enter_context(tc.tile_pool(name="consts", bufs=1))
        dramp = cstack.enter_context(tc.tile_pool(name="dramp", bufs=1, space="DRAM"))

        # ---- persistent constants (few, bundled DMAs; critical first) ----
        identb_sb = consts.tile([128, 128], BF16, tag="identb", name="identb_sb")
        dma(identb_sb[:], identb[:, :])
        hgt_sb = consts.tile([128, 2, NG], BF16, tag="hgt", name="hgt_sb")
        nc.sync.dma_start(out=hgt_sb[:, 0, :], in_=hgt[:, 0, :])
        nc.gpsimd.dma_start(out=hgt_sb[:, 1, :], in_=hgt[:, 1, :])
        wq_sb = consts.tile([128, 2, D], BF16, tag="wq", name="wq_sb")
        dma(wq_sb[:], wqt[:, :, :])
        kt_sb = consts.tile([128, 2, P], F32R, tag="kt", name="kt_sb")
        dma(kt_sb[:], kt[:, :, :])
        hgb_sb = consts.tile([128, NT, D], F32, tag="hgb", name="hgb_sb")
        nc.sync.dma_start(out=hgb_sb[:, 0:3, :], in_=hgb[:, 0:3, :])
        nc.gpsimd.dma_start(out=hgb_sb[:, 3:NT, :], in_=hgb[:, 3:NT, :])
        w1_sb = consts.tile([128, 2, F1], BF16, tag="w1", name="w1_sb")
        dma(w1_sb[:], w1t[:, :, :])
        w2_sb = consts.tile([128, 8, D], BF16, tag="w2", name="w2_sb")
        dma(w2_sb[:], w2r[:, :, :])
        vo_sb = consts.tile([128, NGRP, D], BF16, tag="vo", name="vo_sb")
        dma(vo_sb[:], vo[:, :, :])
        m01_sb = consts.tile([128, H, 128], BF16, tag="m01", name="m01_sb")
        dma(m01_sb[:], m01h[:, :, :])
        rep_sb = consts.tile([128, max(npair, 1), 128], BF16, tag="rep", name="rep_sb")
        dma(rep_sb[:], repm[:, :, :])
        empty_sb = consts.tile([128, 1], F32, tag="empty", name="empty_sb")
        dma(empty_sb[:], emptyp[:, :])
        b1_sb = consts.tile([128, 8, 1], F32, tag="b1c", name="b1_sb")
        dma(b1_sb[:], b1_col[:, :, :])
        bq_sb = None
        if use_bq:
            bq_sb = consts.tile([128, 2, 1], F32, tag="bqc", name="bq_sb")
            dma(bq_sb[:], bq_col[:, :, :])
        ln1_sb = None
        if use_g1 or use_b1ln:
            ln1_sb = consts.tile([128, 2, 2], F32, tag="ln1c", name="ln1_sb")
            dma(ln1_sb[:], ln1c[:, :, :])
        gbr_sb = None
        if use_g2 or use_b2ln or use_b2 or use_g1 or use_b1ln:
            gbr_sb = consts.tile([128, 6, D], F32, tag="gbr", name="gbr_sb")
            nc.gpsimd.dma_start(out=gbr_sb[:], in_=gb_row[:, :].to_broadcast((128, 6, D)))

        # persistent activations
        qT_sb = consts.tile([128, 2, NG], F32R, tag="qT", name="qT_sb")
        Et = consts.tile([128, H, NG], BF16, tag="Et", name="Et")
        Eg = consts.tile([128, NGRP, NG], BF16, tag="Eg", name="Eg")
        den_sb = consts.tile([128, NG], BF16, tag="den", name="den_sb")
        denf_sb = consts.tile([128, NG], F32, tag="denf", name="denf_sb")
        denr_sb = consts.tile([128, NG], BF16, tag="denr", name="denr_sb")
        n_stage = NGRP - n_early
        stg = dramp.tile([max(n_stage, 1), H, GW, NG], BF16, tag="stg", name="stg")

        NCH = 2
        CH = NG // NCH  # 384

        # ================= Phase A: shared projections ==================
        with tc.tile_pool(name="psA", bufs=1, space="PSUM") as psA:
            # PE warmup: ramp the tensor-engine p-state while DMAs land
            wtile = psA.tile([128, NG], BF16, tag="wt", name="wtile")
            for w in range(12):
                nc.tensor.transpose(wtile[:, 0:128], identb_sb[:], identb_sb[:])

            # qT [D, NG] = Wq^T Hg^T  (PSUM writes split at the bank edge)
            for m in range(2):
                ps = psA.tile([128, NG], F32, tag=f"qs{m % 2}", name="ps_q")
                for kk in range(2):
                    for lo, hi in ((0, 512), (512, NG)):
                        nc.tensor.matmul(
                            ps[:, lo:hi], wq_sb[:, kk, m * 128 : (m + 1) * 128],
                            hgt_sb[:, kk, lo:hi], start=(kk == 0), stop=(kk == 1))
                if use_bq:
                    nc.scalar.activation(qT_sb[:, m, :], ps[:], AF.Identity,
                                         bias=bq_sb[:, m, 0:1])
                else:
                    nc.scalar.activation(qT_sb[:, m, :], ps[:], AF.Copy)

            # Et[p, h, n] = exp(s * k_h q_h^T) per head, with the den
            # accumulation matmuls and the early-group regroup DMAs
            # interleaved so the PE never drains during the exp chain.
            psd = [psA.tile([128, CH], F32, tag=f"dn{c}", name=f"psd{c}")
                   for c in range(NCH)]

            def den_mm(h):
                for c in range(NCH):
                    nc.tensor.matmul(
                        psd[c][:], m01_sb[:, h, :],
                        Et[:, h, c * CH : (c + 1) * CH],
                        start=(h == 0), stop=(h == H - 1))

            for h in range(H):
                ps = psA.tile([128, NG], F32, tag=f"qs{h % 2}", name="ps_s")
                for lo, hi in ((0, 512), (512, NG)):
                    nc.tensor.matmul(
                        ps[:, lo:hi],
                        kt_sb[(h % 4) * DH : (h % 4 + 1) * DH, h // 4, :],
                        qT_sb[(h % 4) * DH : (h % 4 + 1) * DH, h // 4, lo:hi],
                        start=True, stop=True,
                        tile_position=((h % 4) * DH, 0))
                nc.scalar.activation(Et[:, h, :], ps[:], AF.Exp, scale=s_attn)
                if h >= 1:
                    den_mm(h - 1)
                for w in range(6):
                    nc.tensor.transpose(wtile[:, 0:128], identb_sb[:],
                                        identb_sb[:])
                for g in range(n_early):
                    dma(Eg[h * GW : (h + 1) * GW, g, :],
                        Et[g * GW : (g + 1) * GW, h, :])
            den_mm(H - 1)

            # den -> reciprocal (fast-approx) -> bf16
            for c in range(NCH):
                nc.vector.tensor_scalar(
                    out=den_sb[:, c * CH : (c + 1) * CH], in0=psd[c][:],
                    scalar1=empty_sb[:, 0:1], scalar2=None, op0=ALU.add)
                nc.vector.reciprocal(
                    out=denr_sb[:, c * CH : (c + 1) * CH],
                    in_=den_sb[:, c * CH : (c + 1) * CH])

            # hold the PE p-state while the reciprocal chain runs
            for w in range(84):
                nc.tensor.transpose(wtile[:, 0:128], identb_sb[:], identb_sb[:])

            # late groups: Et -> DRAM staging -> Eg block layout
            for si in range(n_stage):
                g = n_early + si
                dma(stg[si].rearrange("h i n -> i h n"),
                    Et[g * GW : (g + 1) * GW, :, :])
            if n_stage:
                nc.sync.dma_start(
                    out=Eg[:, n_early:NGRP, :],
                    in_=stg[:].rearrange("g h i n -> (h i) g n"))

        # ================= Phase B: per-batch pipeline ==================
        work = cstack.enter_context(tc.tile_pool(name="work", bufs=2))
        ehpool = cstack.enter_context(tc.tile_pool(name="ehp", bufs=6))
        ps_f1 = cstack.enter_context(tc.tile_pool(name="ps_f1", bufs=2, space="PSUM"))
        ps_m = cstack.enter_context(tc.tile_pool(name="ps_m", bufs=2, space="PSUM"))
        ps_y = cstack.enter_context(tc.tile_pool(name="ps_y", bufs=2, space="PSUM"))

        RC2, RC1, RC0 = 0.29333931447269, -1.1711876763158582, 1.8939170369253155

        def rsqrt_group(vv_ap, mu_ap, sc_ap, ng_ap, sA, sB, eng=None):
            """sc = rsqrt(vv), ng = -mu*sc. Quadratic seed + one Newton step.
            All-DVE by default (cross-engine hops cost semaphore latency);
            with eng=gpsimd the tensor_tensor ops go to Pool (immediate
            tensor_scalar is DVE-only in this toolchain) for chains with
            timing slack."""
            e = eng if eng is not None else nc.vector
            nc.vector.tensor_scalar(out=sA, in0=vv_ap, scalar1=RC2,
                                    scalar2=RC1, op0=ALU.mult, op1=ALU.add)
            e.tensor_tensor(out=sA, in0=sA, in1=vv_ap, op=ALU.mult)
            nc.vector.tensor_scalar(out=sA, in0=sA, scalar1=RC0,
                                    scalar2=None, op0=ALU.add)
            e.tensor_tensor(out=sB, in0=sA, in1=sA, op=ALU.mult)
            e.tensor_tensor(out=sB, in0=sB, in1=vv_ap, op=ALU.mult)
            nc.vector.tensor_scalar(out=sB, in0=sB, scalar1=-0.5,
                                    scalar2=1.5, op0=ALU.mult, op1=ALU.add)
            e.tensor_tensor(out=sc_ap, in0=sA, in1=sB, op=ALU.mult)
            nc.vector.scalar_tensor_tensor(
                out=ng_ap, in0=mu_ap, scalar=-1.0, in1=sc_ap,
                op0=ALU.mult, op1=ALU.mult)

        def new_state(b):
            return {
                "b": b,
                "mvb": work.tile([128, NT, 2], F32, tag="mvb", name="mvb"),
                "sc1": work.tile([128, NT], F32, tag="sc1", name="sc1"),
                "ng1": work.tile([128, NT], F32, tag="ng1", name="ng1"),
                "stats": work.tile([128, 2, 6], F32, tag="stats", name="stats"),
                "xpre": work.tile([128, NT, D], BF16, tag="xpre", name="xpre"),
                "x_row": work.tile([128, NT, D], BF16, tag="x_row", name="x_row",
                                   bufs=3),
                "xT": work.tile([128, 2, NG], BF16, tag="xT", name="xT"),
                "sA": work.tile([128, 6], F32, tag="sA", name="sA"),
                "sB": work.tile([128, 6], F32, tag="sB", name="sB"),
                "eh": [],
            }

        def emit_rep(st):
            """Per-(batch,group) masked replication matmul (PE) + normalize
            multiply (DVE): eh = E_g * masked-replicated reciprocal."""
            b = st["b"]
            if b is None:
                return
            for g, ri in blocks[b]:
                tl = ehpool.tile([128, NG], BF16, tag="eh", name="eh")
                for c in range(NCH):
                    psr = ps_m.tile([128, CH], F32, tag="m", name="psr")
                    nc.tensor.matmul(
                        psr[:], rep_sb[:, ri, :],
                        denr_sb[:, c * CH : (c + 1) * CH],
                        start=True, stop=True)
                    nc.vector.tensor_mul(
                        tl[:, c * CH : (c + 1) * CH],
                        Eg[:, g, c * CH : (c + 1) * CH], psr[:])
                st["eh"].append((g, tl[:]))

        def emit_xp(st):
            """x transposes (PE) + xT evac (split ACT/DVE), per tile."""
            for t in range(NT):
                psx = ps_m.tile([128, D], BF16, tag="m", name="psx")
                for k in range(2):
                    nc.tensor.transpose(
                        psx[:, k * 128 : (k + 1) * 128],
                        st["x_row"][:, t, k * 128 : (k + 1) * 128],
                        identb_sb[:])
                if t < 4:
                    nc.scalar.activation(
                        st["xT"][:, :, t * 128 : (t + 1) * 128],
                        psx[:].rearrange("p (k n) -> p k n", k=2), AF.Copy)
                else:
                    nc.vector.tensor_copy(
                        out=st["xT"][:, :, t * 128 : (t + 1) * 128],
                        in_=psx[:].rearrange("p (k n) -> p k n", k=2))

        def emit_ffn1_m(st, m):
            """One FFN1 m-tile: PE matmuls + ACT gelu."""
            if m == 0:
                st["h1g"] = work.tile([128, 8, NG], BF16, tag="h1g", name="h1g")
            xT = st["xT"]
            ps = ps_f1.tile([128, NG], F32, tag="f1", name="f1")
            for kk in range(2):
                nc.tensor.matmul(ps[:, 0:512], w1_sb[:, kk, m * 128 : (m + 1) * 128],
                                 xT[:, kk, 0:512],
                                 start=(kk == 0), stop=(kk == 1))
                nc.tensor.matmul(ps[:, 512:NG], w1_sb[:, kk, m * 128 : (m + 1) * 128],
                                 xT[:, kk, 512:NG],
                                 start=(kk == 0), stop=(kk == 1))
            if use_b1:
                nc.scalar.activation(st["h1g"][:, m, :], ps[:], AF.Gelu,
                                     bias=b1_sb[:, m, 0:1])
            else:
                nc.scalar.activation(st["h1g"][:, m, :], ps[:], AF.Gelu)

        def emit_front_t(st, t):
            """One front tile: ctx matmuls (PE) + H residual evac (DVE);
            LN1 stats (DVE) in tile pairs."""
            b = st["b"]
            if b is not None:
                psa = ps_m.tile([128, D], F32, tag="m", name="ao")
                nb = len(st["eh"])
                for i, (g, eh_ap) in enumerate(st["eh"]):
                    nc.tensor.matmul(
                        psa[:], eh_ap[:, t * 128 : (t + 1) * 128],
                        vo_sb[:, g, :], start=(i == 0), stop=(i == nb - 1))
                nc.vector.scalar_tensor_tensor(
                    out=st["xpre"][:, t, :], in0=psa[:], scalar=1.0,
                    in1=hgb_sb[:, t, :], op0=ALU.mult, op1=ALU.add)
            src_t = st["xpre"] if b is not None else hgb_sb
            nc.vector.bn_stats(out=st["stats"][:, 0, :], in_=src_t[:, t, :])
            nc.vector.bn_aggr(out=st["mvb"][:, t, :], in_=st["stats"][:, 0, :])

        def emit_front_rsqrt(st, half):
            lo, hi = (0, 2) if half == 0 else (2, 6)
            mvb, sc1, ng1 = st["mvb"], st["sc1"], st["ng1"]
            rsqrt_group(mvb[:, lo:hi, 1], mvb[:, lo:hi, 0],
                        sc1[:, lo:hi], ng1[:, lo:hi],
                        st["sA"][:, lo:hi], st["sB"][:, lo:hi])

        def emit_front_apply(st, half):
            """LN1 applies (Pool tensor_scalar, per-partition AP scalars)."""
            b = st["b"]
            sc1, ng1 = st["sc1"], st["ng1"]
            src = st["xpre"] if b is not None else hgb_sb
            for t in (range(0, 2) if half == 0 else range(2, 6)):
                nc.gpsimd.tensor_scalar(
                    out=st["x_row"][:, t, :], in0=src[:, t, :],
                    scalar1=sc1[:, t : t + 1], scalar2=ng1[:, t : t + 1],
                    op0=ALU.mult, op1=ALU.add)

        def emit_ffn2_ln2(st):
            """FFN2 in row layout (PE, t-outer k-inner) + y residual evac
            (DVE) + LN2 stats (DVE)."""
            st["ypre"] = work.tile([128, NT, D], BF16, tag="ypre", name="ypre")
            st["mv2"] = work.tile([128, NT, 2], F32, tag="mv2", name="mv2")
            st["st2"] = work.tile([128, 2, 6], F32, tag="st2", name="st2")
            h1g = st["h1g"]
            xres = st["x_row"]
            if use_g1 or use_b1ln:
                # residual needs the true x = x_norm*g1 + be1 (g1 folded into
                # W1 elsewhere); rare flag path
                xres = work.tile([128, NT, D], BF16, tag="xres", name="xres")
                for t in range(NT):
                    nc.vector.tensor_mul(xres[:, t, :], st["x_row"][:, t, :],
                                         gbr_sb[:, 4, :])
                    nc.vector.tensor_add(xres[:, t, :], xres[:, t, :],
                                         gbr_sb[:, 5, :])
            for t in range(NT):
                psy = ps_y.tile([128, D], F32, tag="y", name="psy")
                for kk in range(8):
                    nc.tensor.matmul(
                        psy[:], h1g[:, kk, t * 128 : (t + 1) * 128],
                        w2_sb[:, kk, :], start=(kk == 0), stop=(kk == 7))
                if use_b2:
                    nc.vector.tensor_add(psy[:], psy[:], gbr_sb[:, 0, :])
                nc.vector.scalar_tensor_tensor(
                    out=st["ypre"][:, t, :], in0=psy[:], scalar=1.0,
                    in1=xres[:, t, :], op0=ALU.mult, op1=ALU.add)
                nc.vector.bn_stats(out=st["st2"][:, 0, :],
                                   in_=st["ypre"][:, t, :])
                nc.vector.bn_aggr(out=st["mv2"][:, t, :],
                                  in_=st["st2"][:, 0, :])

        def emit_ln2_store(st):
            """LN2 rsqrt (DVE+Pool) + apply (Pool) + store."""
            b = st["b"]
            sc2 = work.tile([128, NT], F32, tag="sc2", name="sc2")
            ng2 = work.tile([128, NT], F32, tag="ng2", name="ng2")
            sA2 = work.tile([128, 6], F32, tag="sA2", name="sA2")
            sB2 = work.tile([128, 6], F32, tag="sB2", name="sB2")
            orow = work.tile([128, NT, D], F32, tag="orow", name="orow")
            ypre, mv2 = st["ypre"], st["mv2"]
            rsqrt_group(mv2[:, :, 1], mv2[:, :, 0], sc2[:, :], ng2[:, :],
                        sA2[:], sB2[:])
            for t in range(NT):
                nc.gpsimd.tensor_scalar(
                    out=orow[:, t, :], in0=ypre[:, t, :],
                    scalar1=sc2[:, t : t + 1], scalar2=ng2[:, t : t + 1],
                    op0=ALU.mult, op1=ALU.add)
                if use_g2:
                    nc.vector.tensor_mul(orow[:, t, :], orow[:, t, :],
                                         gbr_sb[:, 2, :])
                if use_b2ln:
                    nc.vector.tensor_add(orow[:, t, :], orow[:, t, :],
                                         gbr_sb[:, 3, :])
            # stores go on the Sync queue only: a DMA trigger on the GpSimd
            # queue delays the latency-critical Pool LN applies behind it
            if b is not None:
                nc.sync.dma_start(
                    out=out[b].rearrange("(t p) d -> p t d", p=128),
                    in_=orow[:])
            else:
                for be in range(B):
                    if int(counts[be]) == 0:
                        nc.sync.dma_start(
                            out=out[be].rearrange("(t p) d -> p t d", p=128),
                            in_=orow[:])

        # ---- software-pipelined emission --------------------------------
        # PE queue per step: xp(bk) | FFN1-m(bk) interleaved with ctx-t(fr)
        # | FFN2(bk).  ACT queue: gelus(bk) | LN1 applies(fr).  The m/t
        # interleave spaces PSUM-slot reuse past the Pool/DVE drain latency
        # so the PE stream never stalls.
        steps = ([None] if any_empty else []) + jobs
        nsteps = len(steps)
        states = {}
        states[0] = new_state(steps[0])
        emit_rep(states[0])
        for s in range(nsteps + 2):
            fr = states.get(s)
            bk = states.get(s - 1)
            tl = states.get(s - 2)
            if bk is not None:
                emit_xp(bk)
            for i in range(8):
                if fr is not None and i < 6:
                    emit_front_t(fr, i)
                    if i == 1:
                        emit_front_rsqrt(fr, 0)
                        emit_front_apply(fr, 0)
                    elif i == 5:
                        emit_front_rsqrt(fr, 1)
                        emit_front_apply(fr, 1)
                if bk is not None:
                    emit_ffn1_m(bk, i)
            if s + 1 < nsteps:
                states[s + 1] = new_state(steps[s + 1])
                emit_rep(states[s + 1])
            if bk is not None:
                emit_ffn2_ln2(bk)
            if tl is not None:
                emit_ln2_store(tl)
                del states[s - 2]
            if s == 0 and fr is not None:
                # fill the unpipelined first-front latency so the PE p-state
                # survives into step 1
                for w in range(36):
                    wt = ps_m.tile([128, D], BF16, tag="m", name="wt")
                    nc.tensor.transpose(wt[:, 0:128], identb_sb[:],
                                        identb_sb[:])

        cstack.close()

    return nc


def kernel(H_genes, perturbation_indices, batch_assignment, batch_size,
           in_proj_w, in_proj_b, out_proj_w, out_proj_b,
           ffn_w1, ffn_b1, ffn_w2, ffn_b2,
           ln1_g, ln1_b, ln2_g, ln2_b):
    import ml_dtypes
    bf16 = ml_dtypes.bfloat16

    Hg = np.ascontiguousarray(np.asarray(H_genes, dtype=np.float32))
    pidx = np.asarray(perturbation_indices).astype(np.int64)
    ba = np.asarray(batch_assignment).astype(np.int64)
    Bs = int(np.asarray(batch_size))
    assert Bs == B, f"kernel hardcodes B=16, got {Bs}"
    assert Hg.shape == (N, D)

    Wq, Wk, Wv = [np.asarray(w, np.float32) for w in np.split(np.asarray(in_proj_w), 3, axis=0)]
    bq, bk, bv = [np.asarray(x, np.float32) for x in np.split(np.asarray(in_proj_b), 3, axis=0)]
    Wo = np.asarray(out_proj_w, np.float32)
    bo = np.asarray(out_proj_b, np.float32)
    W1 = np.asarray(ffn_w1, np.float32)
    b1 = np.asarray(ffn_b1, np.float32)
    W2 = np.asarray(ffn_w2, np.float32)
    b2 = np.asarray(ffn_b2, np.float32)
    g1 = np.asarray(ln1_g, np.float32)
    be1 = np.asarray(ln1_b, np.float32)
    g2 = np.asarray(ln2_g, np.float32)
    be2 = np.asarray(ln2_b, np.float32)

    counts = np.bincount(ba, minlength=B).astype(np.int64)
    has_any = counts > 0

    # host-side small projections: k and Wo-folded values
    Hp = Hg[pidx]                                   # [P, D]
    k = Hp @ Wk.T + bk[None, :]                     # [P, D]
    v = Hp @ Wv.T + bv[None, :]                     # [P, D]
    # vo[(h,p),:] = v[p, h-slice] @ Wo[:, h-slice].T  (full attn_out proj)
    voA = np.zeros((NGRP, 128, D), np.float32)
    for g in range(NGRP):
        for h in range(H):
            vh = v[g * GW : (g + 1) * GW, h * DH : (h + 1) * DH]   # [16, 32]
            voA[g, h * GW : (h + 1) * GW, :] = vh @ Wo[:, h * DH : (h + 1) * DH].T

    # per-head den stationaries: m01h[h][p, h*16+b] = 1{ba[p]==b}
    m01hA = np.zeros((H, 128, 128), np.float32)
    for h in range(H):
        for p in range(P):
            m01hA[h, p, h * GW + ba[p]] = 1.0

    # per-(batch, group) masked replication matrices:
    # rep[(h,b), (h,i)] = 1{ba[g*16+i] == b}
    blocks = {b: [] for b in range(B)}
    rep_mats = []
    for b in range(B):
        if counts[b] == 0:
            continue
        for g in range(NGRP):
            sel = ba[g * GW : (g + 1) * GW] == b
            if not sel.any():
                continue
            R = np.zeros((128, 128), np.float32)
            for h in range(H):
                for i in range(GW):
                    if sel[i]:
                        R[h * GW + b, h * GW + i] = 1.0
            blocks[b].append((g, len(rep_mats)))
            rep_mats.append(R)
    npair = len(rep_mats)

    # groups needed by the first two jobs get direct (overlapped) regroup
    jobs_order = [b for b in range(B) if counts[b] > 0]
    early_gs = [g for b in jobs_order[:2] for (g, _) in blocks[b]]
    n_early = (max(early_gs) + 1) if early_gs else 0

    # fold ln1 gain into FFN1 (exact): W1' = W1*g1, b1' = W1@b1_ln + b1
    W1f = W1 * g1[None, :]
    b1f = b1 + W1 @ be1

    Hg_pad = np.zeros((NPAD, D), np.float32)
    Hg_pad[:N] = Hg
    emptypA = np.tile((~has_any).astype(np.float32), H)[:, None]  # [(h,b),1]

    flags = (
        bool(np.any(bq != 0)), bool(np.any(b1f != 0)), bool(np.any(b2 != 0)),
        bool(np.any(g1 != 1)), bool(np.any(be1 != 0)),
        bool(np.any(g2 != 1)), bool(np.any(be2 != 0)),
    )
    use_bo = bool(np.any(bo != 0))

    nc = _build_program(counts, blocks, npair, n_early, flags)

    def tile128(a, inner):
        """[K*128, inner...] -> [128, K, inner...] partition-major."""
        a = np.ascontiguousarray(a)
        kdim = a.shape[0] // 128
        return np.ascontiguousarray(
            a.reshape(kdim, 128, *a.shape[1:]).transpose(
                1, 0, *range(2, a.ndim + 1)))

    common = {
        "kt": tile128(k.T, P).astype(np.float32),
        "wqt": tile128(Wq.T, D).astype(bf16),
        "bq_col": tile128(bq[:, None], 1),
        "vo": np.ascontiguousarray(voA.transpose(1, 0, 2)).astype(bf16),
        "m01h": np.ascontiguousarray(m01hA.transpose(1, 0, 2)).astype(bf16),
        "repm": (np.stack(rep_mats).transpose(1, 0, 2) if npair
                 else np.zeros((128, 1, 128), np.float32)).astype(bf16),
        "identb": np.eye(128, dtype=np.float32).astype(bf16),
        "emptyp": np.ascontiguousarray(emptypA),
        "w1t": tile128(W1f.T, F1).astype(bf16),
        "w2r": tile128(W2.T, D).astype(bf16),
        "b1_col": tile128(b1f[:, None], 1),
        "ln1c": tile128(np.stack([g1, be1], axis=1), 2),
        "gb_row": np.stack([b2, be1, g2, be2, g1, be1], axis=0),
    }
    in_maps = []
    for c in range(NCORES):
        sl = Hg_pad[c * NG : (c + 1) * NG]
        m = dict(common)
        hgb = sl + bo[None, :] if use_bo else sl
        m["hgb"] = tile128(hgb, D)
        m["hgt"] = tile128(np.ascontiguousarray(sl.T), NG).astype(bf16)
        in_maps.append(m)

    if os.environ.get("BASS_KERNEL_SIM"):
        from concourse import bass_interp
        # CoreSim lacks a Gelu LUT; shim exact (erf) gelu for local debugging.
        if not getattr(bass_interp.InstructionExecutor, "_gelu_patched", False):
            from scipy.special import erf
            _orig_act = bass_interp.InstructionExecutor.visit_InstActivation

            def _act(self, instruction, *, reg_snapshot=None):
                if instruction.func == mybir.ActivationFunctionType.Gelu:
                    instruction.func = mybir.ActivationFunctionType.Identity
                    try:
                        import concourse.bass_interp as bi
                        out_ap = instruction.outs[0]
                        r = _orig_act(self, instruction, reg_snapshot=reg_snapshot)
                        view = self.view_ap(out_ap, bi.Direction.READ, instruction,
                                            reg_snapshot=reg_snapshot)
                        x = view.astype(np.float64)
                        view[:] = (0.5 * x * (1.0 + erf(x / np.sqrt(2.0)))).astype(view.dtype)
                        return r
                    finally:
                        instruction.func = mybir.ActivationFunctionType.Gelu
                return _orig_act(self, instruction, reg_snapshot=reg_snapshot)

            bass_interp.InstructionExecutor.visit_InstActivation = _act
            bass_interp.InstructionExecutor._gelu_patched = True
        nsim = int(os.environ.get("BASS_KERNEL_SIM_CORES", "1"))
        simtrace = bool(os.environ.get("BASS_KERNEL_SIMTRACE"))
        sim = bass_interp.MultiCoreSim(nc, nsim, trace=simtrace)
        for c in range(nsim):
            for kk, vv in in_maps[c].items():
                sim.cores[c].tensor(kk)[:] = vv
        sim.simulate()
        print(f"SIM predicted time: {sim.cores[0].time} ns")
        full = np.zeros((B, NPAD, D), np.float32)
        for c in range(nsim):
            full[:, c * NG : (c + 1) * NG, :] = (
                np.array(sim.cores[c].mem_tensor("out")).reshape(B, NG, D))
        return full[:, :N, :]

    from concourse.bass_utils import run_bass_kernel_spmd
    _split_waits(nc)
    trace = bool(os.environ.get("BASS_KERNEL_TRACE"))
    res = run_bass_kernel_spmd(nc, in_maps, core_ids=list(range(NCORES)),
                               trace=trace)
    if trace and res.exec_time_ns is not None:
        print(f"HW exec time: {res.exec_time_ns} ns")
        if res.instructions_and_trace:
            print("trace:", res.instructions_and_trace[1])

    full = np.zeros((B, NPAD, D), np.float32)
    for c in range(NCORES):
        full[:, c * NG : (c + 1) * NG, :] = res.results[c]["out"]
    return full[:, :N, :]
